# revision 13
# baseline (speedup 1.0000x reference)
"""TRN2 Bass kernel for nn_Attention_1709396984084.

Reference computation (per batch element b, 8 of them -> one NeuronCore each):
    x  = x_b @ lin_w.T + lin_b          # [S, D]
    Q  = x @ W_q ; K = x @ W_k ; V = x @ W_v
    I  = Q @ K.T  (causal masked, NO 1/sqrt(d) scaling)
    F  = softmax(I) @ V
    returns (F, stack([K, V]))

Key algebraic fold (host side): xp is not an output, so
    Q = x_b @ (lin_w.T @ W_q) + (lin_b @ W_q) = x_b @ Wq' + bq
and likewise for K, V — the linear stage disappears from the device
entirely (Wq'/Wk'/Wv' and the bias vectors are precomputed in float64
on the host).

Device layout (hardcoded for S=2048, D=H=1024, B=8, batch-parallel on 8 cores):
  - host passes xT = x_b.T [D, S]; projections contract over d directly:
    Q^T/K^T [h, s] use Wq' chunks as stationary, xT as moving; V [s, h]
    uses xT chunks as stationary, Wv' as moving.
  - Q^T spills to DRAM, streamed back per 128-query chunk; K^T f32 and
    V bf16 stay resident in SBUF.
  - scores = Q^T.T @ K^T land [q, k] in PSUM; row softmax = free-dim
    reduce_max + ScalarE exp (per-partition -max bias, fused row-sum via
    accum_out); P~ cast bf16, transposed 128x128 on TensorE, then
    P~^T @ V accumulates F; 1/rowsum applied on the way out.
  - matmul dtypes: float32r (fp32 storage, ~1.5e-4 matmul rel err,
    bf16-rate at N=512) for the logit-sensitive chain; bf16 for P@V.
  - attention runs q-chunks DESCENDING with a one-chunk software pipeline
    (two-chunk for the small tail) so P@V fills the PE during softmax.
Measured: relF ~2.8e-3, relK/V ~2e-4 vs the fp32 reference.
"""

import sys

sys.path.insert(0, "/opt/trn_rl_repo")

import numpy as np

P = 128
S = 2048  # sequence length
D = 1024  # input size
H = 1024  # hidden size
FT = 512  # free-dim tile (one PSUM bank of fp32)
NB = 8  # batch == number of cores
DC = D // P  # 8 contraction chunks
HC = H // P
ST = S // FT  # 4 s-tiles
QC = S // P  # 16 query chunks
NEG = -1.0e30

_cache = {}


def _build():
    import concourse.mybir as mybir
    import concourse.tile as tile
    from concourse import bacc
    from concourse.masks import make_identity

    f32 = mybir.dt.float32
    f32r = mybir.dt.float32r
    bf16 = mybir.dt.bfloat16
    EXP = mybir.ActivationFunctionType.Exp
    AX = mybir.AxisListType.X

    nc = bacc.Bacc(None, target_bir_lowering=False)

    xT_d = nc.declare_dram_parameter("xT", [D, S], f32r, isOutput=False)
    wq_d = nc.declare_dram_parameter("Wq", [D, H], f32r, isOutput=False)
    wk_d = nc.declare_dram_parameter("Wk", [D, H], f32r, isOutput=False)
    wv_d = nc.declare_dram_parameter("Wv", [D, H], f32r, isOutput=False)
    bq_d = nc.declare_dram_parameter("bq", [P, HC], f32, isOutput=False)
    bk_d = nc.declare_dram_parameter("bk", [P, HC], f32, isOutput=False)
    bv_d = nc.declare_dram_parameter("bv", [P, H], f32, isOutput=False)
    mask_d = nc.declare_dram_parameter("masks", [4, P, FT], f32, isOutput=False)
    F_d = nc.declare_dram_parameter("F_out", [S, H], f32, isOutput=True)
    KT_d = nc.declare_dram_parameter("KT_out", [H, S], f32r, isOutput=True)
    V_d = nc.declare_dram_parameter("V_out", [S, H], f32r, isOutput=True)
    qt_spill = nc.dram_tensor("QT_spill", [H, S], f32r)

    with tile.TileContext(nc) as tc:
        qtp = tc.alloc_tile_pool(name="qtp", bufs=2)
        biasp = tc.alloc_tile_pool(name="biasp", bufs=1)
        stg = tc.alloc_tile_pool(name="stg", bufs=3)
        xtp = tc.alloc_tile_pool(name="xtp", bufs=1)
        wchp = tc.alloc_tile_pool(name="wchp", bufs=4)
        psmm = tc.alloc_tile_pool(name="psmm", bufs=8, space="PSUM")

        xt_sb = xtp.tile([P, DC, S], f32r, tag="xt", name="xt")
        bqk_sb = biasp.tile([P, 2, HC], f32, tag="bqk", name="bqk")
        bq_sb = bqk_sb[:, 0]
        bk_sb = bqk_sb[:, 1]
        bv_sb = biasp.tile([P, H], f32, tag="bv", name="bv")
        nc.sync.dma_start(bq_sb[:], bq_d.ap())
        nc.sync.dma_start(bk_sb[:], bk_d.ap())
        nc.sync.dma_start(bv_sb[:], bv_d.ap())
        wvp = tc.alloc_tile_pool(name="wvp", bufs=1, side="right")
        wv_sb = wvp.tile([P, DC, H], f32r, tag="wv", name="wv")
        wqp = tc.alloc_tile_pool(name="wqp", bufs=1)
        wq_sb = wqp.tile([P, DC, H], f32r, tag="wq", name="wq")
        # issue order = need order: xt st0, then Wq' (hc-major), then the rest
        # of xt, then Wv' (used ~100us later)
        for dc in range(DC):
            nc.sync.dma_start(
                xt_sb[:, dc, 0:FT], xT_d.ap()[dc * P : (dc + 1) * P, 0:FT]
            )
        for hc in range(HC):
            for dc in range(DC):
                nc.sync.dma_start(
                    wq_sb[:, dc, hc * P : (hc + 1) * P],
                    wq_d.ap()[dc * P : (dc + 1) * P, hc * P : (hc + 1) * P],
                )
        for st in range(1, ST):
            for dc in range(DC):
                nc.sync.dma_start(
                    xt_sb[:, dc, st * FT : (st + 1) * FT],
                    xT_d.ap()[dc * P : (dc + 1) * P, st * FT : (st + 1) * FT],
                )
        for ec in range(DC):
            nc.sync.dma_start(
                wv_sb[:, ec, 0:FT], wv_d.ap()[ec * P : (ec + 1) * P, 0:FT]
            )
            nc.sync.dma_start(
                wv_sb[:, ec, FT:H], wv_d.ap()[ec * P : (ec + 1) * P, FT:H]
            )

        # ---- Q^T -> DRAM spill; st-outer + resident Wq' for early start ----
        for st in range(ST):
            for hc in range(HC):
                pt = psmm.tile([P, FT], f32, tag="mm", name="mm")
                for dc in range(DC):
                    nc.tensor.matmul(
                        pt[:],
                        wq_sb[:, dc, hc * P : (hc + 1) * P],
                        xt_sb[:, dc, st * FT : (st + 1) * FT],
                        start=(dc == 0),
                        stop=(dc == DC - 1),
                    )
                qstg = stg.tile([P, FT], f32r, tag="stg", name="stg")
                nc.vector.tensor_add(
                    qstg[:], pt[:], bq_sb[:, hc : hc + 1].to_broadcast((P, FT))
                )
                nc.sync.dma_start(
                    qt_spill.ap()[hc * P : (hc + 1) * P, st * FT : (st + 1) * FT],
                    qstg[:],
                )
        wqp.release()

        # prefetch the first two attention q-chunks' Q^T columns now
        qt_tiles = {}

        def load_qt(qi):
            qt = qtp.tile([P, HC, P], f32r, tag="qt", name="qt")
            for hc in range(HC):
                nc.sync.dma_start(
                    qt[:, hc, :],
                    qt_spill.ap()[hc * P : (hc + 1) * P, qi * P : (qi + 1) * P],
                )
            qt_tiles[qi] = qt

        load_qt(QC - 1)
        load_qt(QC - 2)

        # ---- K^T resident + K cache out (streamed Wk' chunks) ----
        ktp = tc.alloc_tile_pool(name="ktp", bufs=1, side="right")
        kt_sb = ktp.tile([P, HC, S], f32r, tag="kt", name="kt")
        wk_tiles = {}

        def load_wch(hc):
            wch = wchp.tile([P, DC, P], f32r, tag="wch", name="wch")
            for ec in range(DC):
                nc.sync.dma_start(
                    wch[:, ec, :],
                    wk_d.ap()[ec * P : (ec + 1) * P, hc * P : (hc + 1) * P],
                )
            wk_tiles[hc] = wch

        load_wch(0)
        load_wch(1)
        load_wch(2)
        for hc in range(HC):
            if hc + 3 < HC:
                load_wch(hc + 3)
            wch = wk_tiles.pop(hc)
            pts = [psmm.tile([P, FT], f32, tag="mm", name="mm") for _ in range(ST)]
            for ec in range(DC):
                for st in range(ST):
                    nc.tensor.matmul(
                        pts[st][:],
                        wch[:, ec, :],
                        xt_sb[:, ec, st * FT : (st + 1) * FT],
                        start=(ec == 0),
                        stop=(ec == DC - 1),
                    )
            for st in range(ST):
                nc.vector.tensor_add(
                    kt_sb[:, hc, st * FT : (st + 1) * FT],
                    pts[st][:],
                    bk_sb[:, hc : hc + 1].to_broadcast((P, FT)),
                )
                nc.sync.dma_start(
                    KT_d.ap()[hc * P : (hc + 1) * P, st * FT : (st + 1) * FT],
                    kt_sb[:, hc, st * FT : (st + 1) * FT],
                )

        # ---- V natural + cache out + bf16 copy ----
        # xT is streamed back from DRAM here so the big resident xt tile can
        # be released before the V stage (SBUF headroom for kt/v_bf).
        wchp.release()
        xtp.release()
        vxtp = tc.alloc_tile_pool(name="vxtp", bufs=6)
        vbfp = tc.alloc_tile_pool(name="vbfp", bufs=1, side="right")
        v_bf = vbfp.tile([P, QC, H], bf16, tag="vbf", name="vbf")
        vxt_tiles = {}

        def load_vxt(sc):
            vxt = vxtp.tile([P, DC, P], f32r, tag="vxt", name="vxt")
            for ec in range(DC):
                nc.sync.dma_start(
                    vxt[:, ec, :],
                    xT_d.ap()[ec * P : (ec + 1) * P, sc * P : (sc + 1) * P],
                )
            vxt_tiles[sc] = vxt

        for sc in range(5):
            load_vxt(sc)
        for sc in range(QC):
            if sc + 5 < QC:
                load_vxt(sc + 5)
            vxt = vxt_tiles.pop(sc)
            pts = [psmm.tile([P, FT], f32, tag="mm", name="mm") for _ in range(2)]
            for ec in range(DC):
                for ht in range(2):
                    nc.tensor.matmul(
                        pts[ht][:],
                        vxt[:, ec, :],
                        wv_sb[:, ec, ht * FT : (ht + 1) * FT],
                        start=(ec == 0),
                        stop=(ec == DC - 1),
                    )
            for ht in range(2):
                vstg = stg.tile([P, FT], f32r, tag="stg", name="stg")
                nc.vector.tensor_add(
                    vstg[:], pts[ht][:], bv_sb[:, ht * FT : (ht + 1) * FT]
                )
                nc.sync.dma_start(
                    V_d.ap()[sc * P : (sc + 1) * P, ht * FT : (ht + 1) * FT],
                    vstg[:],
                )
                nc.scalar.copy(v_bf[:, sc, ht * FT : (ht + 1) * FT], vstg[:])

        # ---- attention, one 128-query chunk at a time, DESCENDING ----
        vxtp.release()
        stg.release()
        psmm.release()
        with (
            tc.tile_pool(name="small", bufs=1) as small,
            tc.tile_pool(name="pbfp", bufs=2) as pbfp,
            tc.tile_pool(name="ptp", bufs=2) as ptp,
            tc.tile_pool(name="fp", bufs=2) as fp,
            tc.tile_pool(name="smp", bufs=3) as smp,
            tc.tile_pool(name="psS", bufs=6, space="PSUM") as psS,
            tc.tile_pool(name="psF", bufs=2, space="PSUM") as psF,
        ):
            mask_sb = small.tile([P, 4, FT], f32, tag="mask", name="mask")
            for v in range(4):
                nc.sync.dma_start(mask_sb[:, v, :], mask_d.ap()[v])
            ident = small.tile([P, P], bf16, tag="ident", name="ident")
            make_identity(nc, ident[:])

            def qk_block(qi):
                n_kt = qi // 4 + 1
                if qi - 2 >= 0:
                    load_qt(qi - 2)
                qt = qt_tiles.pop(qi)
                sts = [
                    psS.tile([P, FT], f32, tag="S", name="S") for _ in range(n_kt)
                ]
                for kt in range(n_kt):
                    for hc in range(HC):
                        nc.tensor.matmul(
                            sts[kt][:],
                            qt[:, hc, :],
                            kt_sb[:, hc, kt * FT : (kt + 1) * FT],
                            start=(hc == 0),
                            stop=(hc == HC - 1),
                        )
                return sts

            def softmax_block(qi, sts):
                n_kt = len(sts)
                v = qi % 4
                nc.vector.tensor_add(sts[-1][:], sts[-1][:], mask_sb[:, v, :])
                sm = smp.tile([P, 16], f32, tag="sm", name="sm")
                for kt in range(n_kt):
                    nc.vector.reduce_max(sm[:, kt : kt + 1], sts[kt][:], axis=AX)
                negm = sm[:, 8:9]
                nc.vector.reduce_max(negm, sm[:, :n_kt], axis=AX, negate=True)
                p_bf = pbfp.tile([P, S], bf16, tag="pbf", name="pbf")
                for kt in range(n_kt):
                    nc.scalar.activation(
                        p_bf[:, kt * FT : (kt + 1) * FT],
                        sts[kt][:],
                        EXP,
                        bias=negm,
                        accum_out=sm[:, 4 + kt : 5 + kt],
                    )
                recip = sm[:, 10:11]
                if n_kt > 1:
                    nc.vector.reduce_sum(sm[:, 9:10], sm[:, 4 : 4 + n_kt], axis=AX)
                    nc.vector.reciprocal(recip, sm[:, 9:10])
                else:
                    nc.vector.reciprocal(recip, sm[:, 4:5])
                ptb = ptp.tile([P, QC, P], bf16, tag="pt", name="pt")
                for kc in range(qi + 1):
                    tp = psS.tile([P, P], bf16, tag="S", name="S_tp")
                    nc.tensor.transpose(
                        tp[:], p_bf[:, kc * P : (kc + 1) * P], ident[:]
                    )
                    nc.vector.tensor_copy(ptb[:, kc, :], tp[:])
                return ptb, recip

            def pv_block(qi, ptb, recip):
                fts = [psF.tile([P, FT], f32, tag="F", name="F") for _ in range(2)]
                for kc in range(qi + 1):
                    for ht in range(2):
                        nc.tensor.matmul(
                            fts[ht][:],
                            ptb[:, kc, :],
                            v_bf[:, kc, ht * FT : (ht + 1) * FT],
                            start=(kc == 0),
                            stop=(kc == qi),
                        )
                fsb = fp.tile([P, H], f32, tag="fsb", name="fsb")
                for ht in range(2):
                    nc.vector.tensor_mul(
                        fsb[:, ht * FT : (ht + 1) * FT],
                        fts[ht][:],
                        recip.to_broadcast((P, FT)),
                    )
                nc.sync.dma_start(F_d.ap()[qi * P : (qi + 1) * P, :], fsb[:])

            pending = None
            for qi in range(QC - 1, 5, -1):
                sts = qk_block(qi)
                ptb, recip = softmax_block(qi, sts)
                if pending is not None:
                    pv_block(*pending)
                pending = (qi, ptb, recip)
            for a in (5, 3, 1):
                b = a - 1
                sts_a = qk_block(a)
                sts_b = qk_block(b)
                ptb_a, recip_a = softmax_block(a, sts_a)
                if pending is not None:
                    pv_block(*pending)
                ptb_b, recip_b = softmax_block(b, sts_b)
                pv_block(a, ptb_a, recip_a)
                pending = (b, ptb_b, recip_b)
            pv_block(*pending)
        vbfp.release()
        ktp.release()
        wvp.release()
        biasp.release()
        qtp.release()

    nc.compile()
    return nc


def _get_nc():
    if "nc" not in _cache:
        _cache["nc"] = _build()
    return _cache["nc"]


def _masks():
    m = np.full((4, P, FT), NEG, dtype=np.float32)
    j = np.arange(FT)[None, :]
    p = np.arange(P)[:, None]
    for v in range(4):
        m[v][j <= p + P * v] = 0.0
    return m


_last_in_maps = None


def kernel(x_batch, lin_w, lin_b, W_q, W_k, W_v):
    from concourse.bass_utils import run_bass_kernel_spmd

    nc = _get_nc()
    x_batch = np.asarray(x_batch, dtype=np.float32)
    lwT64 = np.asarray(lin_w, dtype=np.float64).T
    lb64 = np.asarray(lin_b, dtype=np.float64)
    cw = {}
    for nm, w in (("q", W_q), ("k", W_k), ("v", W_v)):
        w64 = np.asarray(w, dtype=np.float64)
        cw["W" + nm] = np.ascontiguousarray((lwT64 @ w64).astype(np.float32))
        cw["b" + nm] = (lb64 @ w64).astype(np.float32)
    bq = np.ascontiguousarray(cw["bq"].reshape(HC, P).T)  # [P, HC]
    bk = np.ascontiguousarray(cw["bk"].reshape(HC, P).T)
    bv = np.ascontiguousarray(np.tile(cw["bv"][None, :], (P, 1)))  # [P, H]
    masks = _masks()

    in_maps = []
    for c in range(NB):
        in_maps.append(
            {
                "xT": np.ascontiguousarray(x_batch[c].T),
                "Wq": cw["Wq"],
                "Wk": cw["Wk"],
                "Wv": cw["Wv"],
                "bq": bq,
                "bk": bk,
                "bv": bv,
                "masks": masks,
            }
        )
    global _last_in_maps
    _last_in_maps = in_maps
    res = run_bass_kernel_spmd(nc, in_maps, core_ids=list(range(NB)))
    F = np.stack([res.results[c]["F_out"] for c in range(NB)])
    K = np.stack([np.ascontiguousarray(res.results[c]["KT_out"].T) for c in range(NB)])
    V = np.stack([res.results[c]["V_out"] for c in range(NB)])
    cache = np.stack([K, V])
    return (F, cache)


# revision 14
# speedup vs baseline: 1.2709x; 1.2709x over previous
"""TRN2 Bass kernel for nn_Attention_1709396984084.

Reference computation (per batch element b, 8 of them -> one NeuronCore each):
    x  = x_b @ lin_w.T + lin_b          # [S, D]
    Q  = x @ W_q ; K = x @ W_k ; V = x @ W_v
    I  = Q @ K.T  (causal masked, NO 1/sqrt(d) scaling)
    F  = softmax(I) @ V
    returns (F, stack([K, V]))

Key algebraic fold (host side): xp is not an output, so
    Q = x_b @ (lin_w.T @ W_q) + (lin_b @ W_q) = x_b @ Wq' + bq
and likewise for K, V — the linear stage disappears from the device
entirely (Wq'/Wk'/Wv' and the bias vectors are precomputed in float64
on the host).

Device layout (hardcoded for S=2048, D=H=1024, B=8, batch-parallel on 8 cores):
  - host passes xT = x_b.T [D, S]; projections contract over d directly:
    Q^T/K^T [h, s] use Wq' chunks as stationary, xT as moving; V [s, h]
    uses xT chunks as stationary, Wv' as moving.
  - Q^T spills to DRAM, streamed back per 128-query chunk; K^T f32 and
    V bf16 stay resident in SBUF.
  - scores = Q^T.T @ K^T land [q, k] in PSUM; row softmax = free-dim
    reduce_max + ScalarE exp (per-partition -max bias, fused row-sum via
    accum_out); P~ cast bf16, transposed 128x128 on TensorE, then
    P~^T @ V accumulates F; 1/rowsum applied on the way out.
  - matmul dtypes: float32r (fp32 storage, ~1.5e-4 matmul rel err,
    bf16-rate at N=512) for the logit-sensitive chain; bf16 for P@V.
  - attention runs q-chunks DESCENDING with a one-chunk software pipeline
    (two-chunk for the small tail) so P@V fills the PE during softmax.
Measured: relF ~2.8e-3, relK/V ~2e-4 vs the fp32 reference.
"""

import sys

sys.path.insert(0, "/opt/trn_rl_repo")

import numpy as np

P = 128
S = 2048  # sequence length
D = 1024  # input size
H = 1024  # hidden size
FT = 512  # free-dim tile (one PSUM bank of fp32)
NB = 8  # batch == number of cores
DC = D // P  # 8 contraction chunks
HC = H // P
ST = S // FT  # 4 s-tiles
QC = S // P  # 16 query chunks
NEG = -1.0e30

_cache = {}


def _build():
    import concourse.mybir as mybir
    import concourse.tile as tile
    from concourse import bacc
    from concourse.masks import make_identity

    f32 = mybir.dt.float32
    f32r = mybir.dt.float32r
    bf16 = mybir.dt.bfloat16
    EXP = mybir.ActivationFunctionType.Exp
    AX = mybir.AxisListType.X

    nc = bacc.Bacc(None, target_bir_lowering=False)

    # all inputs pre-swizzled on the host so every DMA is long-contiguous
    # per SBUF partition (512B-run chunked loads were DMA-descriptor-bound)
    xT_d = nc.declare_dram_parameter("xT", [P, DC, S], f32r, isOutput=False)
    xv_d = nc.declare_dram_parameter("xv", [P, QC, DC, P], f32r, isOutput=False)
    wq_d = nc.declare_dram_parameter("Wq", [P, DC, H], f32r, isOutput=False)
    wk_d = nc.declare_dram_parameter("Wk", [P, HC, DC, P], f32r, isOutput=False)
    wv_d = nc.declare_dram_parameter("Wv", [P, DC, H], f32r, isOutput=False)
    bq_d = nc.declare_dram_parameter("bq", [P, HC], f32, isOutput=False)
    bk_d = nc.declare_dram_parameter("bk", [P, HC], f32, isOutput=False)
    bv_d = nc.declare_dram_parameter("bv", [P, H], f32, isOutput=False)
    mask_d = nc.declare_dram_parameter("masks", [4, P, FT], f32, isOutput=False)
    F_d = nc.declare_dram_parameter("F_out", [S, H], f32, isOutput=True)
    KT_d = nc.declare_dram_parameter("KT_out", [H, S], f32r, isOutput=True)
    V_d = nc.declare_dram_parameter("V_out", [S, H], f32r, isOutput=True)
    qt_spill = nc.dram_tensor("QT_spill", [QC, P, HC, P], f32r)

    with tile.TileContext(nc) as tc:
        qtp = tc.alloc_tile_pool(name="qtp", bufs=2)
        biasp = tc.alloc_tile_pool(name="biasp", bufs=1)
        stg = tc.alloc_tile_pool(name="stg", bufs=3)
        xtp = tc.alloc_tile_pool(name="xtp", bufs=1)
        wchp = tc.alloc_tile_pool(name="wchp", bufs=4)
        psmm = tc.alloc_tile_pool(name="psmm", bufs=8, space="PSUM")

        xt_sb = xtp.tile([P, DC, S], f32r, tag="xt", name="xt")
        bqk_sb = biasp.tile([P, 2, HC], f32, tag="bqk", name="bqk")
        bq_sb = bqk_sb[:, 0]
        bk_sb = bqk_sb[:, 1]
        bv_sb = biasp.tile([P, H], f32, tag="bv", name="bv")
        nc.sync.dma_start(bq_sb[:], bq_d.ap())
        nc.sync.dma_start(bk_sb[:], bk_d.ap())
        nc.sync.dma_start(bv_sb[:], bv_d.ap())
        wvp = tc.alloc_tile_pool(name="wvp", bufs=1, side="right")
        wv_sb = wvp.tile([P, DC, H], f32r, tag="wv", name="wv")
        wqp = tc.alloc_tile_pool(name="wqp", bufs=1)
        wq_sb = wqp.tile([P, DC, H], f32r, tag="wq", name="wq")
        # issue order = need order: xt st0, then Wq', then the rest of xt,
        # then Wv' (used ~100us later)
        for dc in range(DC):
            nc.sync.dma_start(xt_sb[:, dc, 0:FT], xT_d.ap()[:, dc, 0:FT])
        for dc in range(DC):
            nc.sync.dma_start(wq_sb[:, dc, :], wq_d.ap()[:, dc, :])
        for st in range(1, ST):
            for dc in range(DC):
                nc.sync.dma_start(
                    xt_sb[:, dc, st * FT : (st + 1) * FT],
                    xT_d.ap()[:, dc, st * FT : (st + 1) * FT],
                )
        for ec in range(DC):
            nc.sync.dma_start(wv_sb[:, ec, :], wv_d.ap()[:, ec, :])

        # ---- Q^T -> DRAM spill; st-outer + resident Wq' for early start ----
        for st in range(ST):
            for hc in range(HC):
                pt = psmm.tile([P, FT], f32, tag="mm", name="mm")
                for dc in range(DC):
                    nc.tensor.matmul(
                        pt[:],
                        wq_sb[:, dc, hc * P : (hc + 1) * P],
                        xt_sb[:, dc, st * FT : (st + 1) * FT],
                        start=(dc == 0),
                        stop=(dc == DC - 1),
                    )
                qstg = stg.tile([P, FT], f32r, tag="stg", name="stg")
                nc.vector.tensor_add(
                    qstg[:], pt[:], bq_sb[:, hc : hc + 1].to_broadcast((P, FT))
                )
                for qj in range(4):
                    nc.sync.dma_start(
                        qt_spill.ap()[st * 4 + qj, :, hc, :],
                        qstg[:, qj * P : (qj + 1) * P],
                    )
        wqp.release()

        # prefetch the first two attention q-chunks' Q^T columns now
        qt_tiles = {}

        def load_qt(qi):
            qt = qtp.tile([P, HC, P], f32r, tag="qt", name="qt")
            nc.sync.dma_start(qt[:, 0 : HC // 2, :], qt_spill.ap()[qi, :, 0 : HC // 2, :])
            nc.sync.dma_start(qt[:, HC // 2 :, :], qt_spill.ap()[qi, :, HC // 2 :, :])
            qt_tiles[qi] = qt

        load_qt(QC - 1)
        load_qt(QC - 2)

        # ---- K^T resident + K cache out (streamed Wk' chunks) ----
        ktp = tc.alloc_tile_pool(name="ktp", bufs=1, side="right")
        kt_sb = ktp.tile([P, HC, S], f32r, tag="kt", name="kt")
        wk_tiles = {}

        def load_wch(hc):
            wch = wchp.tile([P, DC, P], f32r, tag="wch", name="wch")
            nc.sync.dma_start(wch[:], wk_d.ap()[:, hc])
            wk_tiles[hc] = wch

        load_wch(0)
        load_wch(1)
        load_wch(2)
        for hc in range(HC):
            if hc + 3 < HC:
                load_wch(hc + 3)
            wch = wk_tiles.pop(hc)
            pts = [psmm.tile([P, FT], f32, tag="mm", name="mm") for _ in range(ST)]
            for ec in range(DC):
                for st in range(ST):
                    nc.tensor.matmul(
                        pts[st][:],
                        wch[:, ec, :],
                        xt_sb[:, ec, st * FT : (st + 1) * FT],
                        start=(ec == 0),
                        stop=(ec == DC - 1),
                    )
            for st in range(ST):
                nc.vector.tensor_add(
                    kt_sb[:, hc, st * FT : (st + 1) * FT],
                    pts[st][:],
                    bk_sb[:, hc : hc + 1].to_broadcast((P, FT)),
                )
                nc.sync.dma_start(
                    KT_d.ap()[hc * P : (hc + 1) * P, st * FT : (st + 1) * FT],
                    kt_sb[:, hc, st * FT : (st + 1) * FT],
                )

        # ---- V natural + cache out + bf16 copy ----
        # xT is streamed back from DRAM here so the big resident xt tile can
        # be released before the V stage (SBUF headroom for kt/v_bf).
        wchp.release()
        xtp.release()
        vxtp = tc.alloc_tile_pool(name="vxtp", bufs=6)
        vbfp = tc.alloc_tile_pool(name="vbfp", bufs=1, side="right")
        v_bf = vbfp.tile([P, QC, H], bf16, tag="vbf", name="vbf")
        vxt_tiles = {}

        def load_vxt(sc):
            vxt = vxtp.tile([P, DC, P], f32r, tag="vxt", name="vxt")
            nc.sync.dma_start(vxt[:], xv_d.ap()[:, sc])
            vxt_tiles[sc] = vxt

        for sc in range(5):
            load_vxt(sc)
        for sc in range(QC):
            if sc + 5 < QC:
                load_vxt(sc + 5)
            vxt = vxt_tiles.pop(sc)
            pts = [psmm.tile([P, FT], f32, tag="mm", name="mm") for _ in range(2)]
            for ec in range(DC):
                for ht in range(2):
                    nc.tensor.matmul(
                        pts[ht][:],
                        vxt[:, ec, :],
                        wv_sb[:, ec, ht * FT : (ht + 1) * FT],
                        start=(ec == 0),
                        stop=(ec == DC - 1),
                    )
            for ht in range(2):
                vstg = stg.tile([P, FT], f32r, tag="stg", name="stg")
                nc.vector.tensor_add(
                    vstg[:], pts[ht][:], bv_sb[:, ht * FT : (ht + 1) * FT]
                )
                nc.sync.dma_start(
                    V_d.ap()[sc * P : (sc + 1) * P, ht * FT : (ht + 1) * FT],
                    vstg[:],
                )
                nc.scalar.copy(v_bf[:, sc, ht * FT : (ht + 1) * FT], vstg[:])

        # ---- attention, one 128-query chunk at a time, DESCENDING ----
        vxtp.release()
        stg.release()
        psmm.release()
        with (
            tc.tile_pool(name="small", bufs=1) as small,
            tc.tile_pool(name="pbfp", bufs=2) as pbfp,
            tc.tile_pool(name="ptp", bufs=2) as ptp,
            tc.tile_pool(name="fp", bufs=2) as fp,
            tc.tile_pool(name="smp", bufs=3) as smp,
            tc.tile_pool(name="psS", bufs=6, space="PSUM") as psS,
            tc.tile_pool(name="psF", bufs=2, space="PSUM") as psF,
        ):
            mask_sb = small.tile([P, 4, FT], f32, tag="mask", name="mask")
            for v in range(4):
                nc.sync.dma_start(mask_sb[:, v, :], mask_d.ap()[v])
            ident = small.tile([P, P], bf16, tag="ident", name="ident")
            make_identity(nc, ident[:])

            def qk_block(qi):
                n_kt = qi // 4 + 1
                if qi - 2 >= 0:
                    load_qt(qi - 2)
                qt = qt_tiles.pop(qi)
                sts = [
                    psS.tile([P, FT], f32, tag="S", name="S") for _ in range(n_kt)
                ]
                for kt in range(n_kt):
                    for hc in range(HC):
                        nc.tensor.matmul(
                            sts[kt][:],
                            qt[:, hc, :],
                            kt_sb[:, hc, kt * FT : (kt + 1) * FT],
                            start=(hc == 0),
                            stop=(hc == HC - 1),
                        )
                return sts

            def softmax_block(qi, sts):
                n_kt = len(sts)
                v = qi % 4
                nc.vector.tensor_add(sts[-1][:], sts[-1][:], mask_sb[:, v, :])
                sm = smp.tile([P, 16], f32, tag="sm", name="sm")
                for kt in range(n_kt):
                    nc.vector.reduce_max(sm[:, kt : kt + 1], sts[kt][:], axis=AX)
                negm = sm[:, 8:9]
                nc.vector.reduce_max(negm, sm[:, :n_kt], axis=AX, negate=True)
                p_bf = pbfp.tile([P, S], bf16, tag="pbf", name="pbf")
                for kt in range(n_kt):
                    nc.scalar.activation(
                        p_bf[:, kt * FT : (kt + 1) * FT],
                        sts[kt][:],
                        EXP,
                        bias=negm,
                        accum_out=sm[:, 4 + kt : 5 + kt],
                    )
                recip = sm[:, 10:11]
                if n_kt > 1:
                    nc.vector.reduce_sum(sm[:, 9:10], sm[:, 4 : 4 + n_kt], axis=AX)
                    nc.vector.reciprocal(recip, sm[:, 9:10])
                else:
                    nc.vector.reciprocal(recip, sm[:, 4:5])
                ptb = ptp.tile([P, QC, P], bf16, tag="pt", name="pt")
                for kc in range(qi + 1):
                    tp = psS.tile([P, P], bf16, tag="S", name="S_tp")
                    nc.tensor.transpose(
                        tp[:], p_bf[:, kc * P : (kc + 1) * P], ident[:]
                    )
                    nc.vector.tensor_copy(ptb[:, kc, :], tp[:])
                return ptb, recip

            def pv_block(qi, ptb, recip):
                fts = [psF.tile([P, FT], f32, tag="F", name="F") for _ in range(2)]
                for kc in range(qi + 1):
                    for ht in range(2):
                        nc.tensor.matmul(
                            fts[ht][:],
                            ptb[:, kc, :],
                            v_bf[:, kc, ht * FT : (ht + 1) * FT],
                            start=(kc == 0),
                            stop=(kc == qi),
                        )
                fsb = fp.tile([P, H], f32, tag="fsb", name="fsb")
                for ht in range(2):
                    nc.vector.tensor_mul(
                        fsb[:, ht * FT : (ht + 1) * FT],
                        fts[ht][:],
                        recip.to_broadcast((P, FT)),
                    )
                nc.sync.dma_start(F_d.ap()[qi * P : (qi + 1) * P, :], fsb[:])

            pending = None
            for qi in range(QC - 1, 5, -1):
                sts = qk_block(qi)
                ptb, recip = softmax_block(qi, sts)
                if pending is not None:
                    pv_block(*pending)
                pending = (qi, ptb, recip)
            for a in (5, 3, 1):
                b = a - 1
                sts_a = qk_block(a)
                sts_b = qk_block(b)
                ptb_a, recip_a = softmax_block(a, sts_a)
                if pending is not None:
                    pv_block(*pending)
                ptb_b, recip_b = softmax_block(b, sts_b)
                pv_block(a, ptb_a, recip_a)
                pending = (b, ptb_b, recip_b)
            pv_block(*pending)
        vbfp.release()
        ktp.release()
        wvp.release()
        biasp.release()
        qtp.release()

    nc.compile()
    return nc


def _get_nc():
    if "nc" not in _cache:
        _cache["nc"] = _build()
    return _cache["nc"]


def _masks():
    m = np.full((4, P, FT), NEG, dtype=np.float32)
    j = np.arange(FT)[None, :]
    p = np.arange(P)[:, None]
    for v in range(4):
        m[v][j <= p + P * v] = 0.0
    return m


_last_in_maps = None


def kernel(x_batch, lin_w, lin_b, W_q, W_k, W_v):
    from concourse.bass_utils import run_bass_kernel_spmd

    nc = _get_nc()
    x_batch = np.asarray(x_batch, dtype=np.float32)
    lwT64 = np.asarray(lin_w, dtype=np.float64).T
    lb64 = np.asarray(lin_b, dtype=np.float64)
    cw = {}
    for nm, w in (("q", W_q), ("k", W_k), ("v", W_v)):
        w64 = np.asarray(w, dtype=np.float64)
        cw["W" + nm] = np.ascontiguousarray((lwT64 @ w64).astype(np.float32))
        cw["b" + nm] = (lb64 @ w64).astype(np.float32)
    bq = np.ascontiguousarray(cw["bq"].reshape(HC, P).T)  # [P, HC]
    bk = np.ascontiguousarray(cw["bk"].reshape(HC, P).T)
    bv = np.ascontiguousarray(np.tile(cw["bv"][None, :], (P, 1)))  # [P, H]
    masks = _masks()

    def sw_dPH(w):  # [D, H] -> [P, DC, H]
        return np.ascontiguousarray(w.reshape(DC, P, H).transpose(1, 0, 2))

    def sw_chunked(w):  # [D, H] -> [P, HC, DC, P]
        return np.ascontiguousarray(
            w.reshape(DC, P, HC, P).transpose(1, 2, 0, 3)
        )

    wq_sw = sw_dPH(cw["Wq"])
    wk_sw = sw_chunked(cw["Wk"])
    wv_sw = sw_dPH(cw["Wv"])
    in_maps = []
    for c in range(NB):
        xb = x_batch[c]  # [S, D]
        xT_sw = np.ascontiguousarray(
            xb.T.reshape(DC, P, S).transpose(1, 0, 2)
        )  # [P, DC, S]
        xv_sw = np.ascontiguousarray(
            xb.reshape(QC, P, DC, P).transpose(3, 0, 2, 1)
        )  # [P, QC, DC, P] : xv[p, sc, dc, j] = x[sc*128+j, dc*128+p]
        in_maps.append(
            {
                "xT": xT_sw,
                "xv": xv_sw,
                "Wq": wq_sw,
                "Wk": wk_sw,
                "Wv": wv_sw,
                "bq": bq,
                "bk": bk,
                "bv": bv,
                "masks": masks,
            }
        )
    global _last_in_maps
    _last_in_maps = in_maps
    res = run_bass_kernel_spmd(nc, in_maps, core_ids=list(range(NB)))
    F = np.stack([res.results[c]["F_out"] for c in range(NB)])
    K = np.stack([np.ascontiguousarray(res.results[c]["KT_out"].T) for c in range(NB)])
    V = np.stack([res.results[c]["V_out"] for c in range(NB)])
    cache = np.stack([K, V])
    return (F, cache)


# revision 15
# speedup vs baseline: 1.4137x; 1.1124x over previous
"""TRN2 Bass kernel for nn_Attention_1709396984084.

Reference computation (per batch element b, 8 of them -> one NeuronCore each):
    x  = x_b @ lin_w.T + lin_b          # [S, D]
    Q  = x @ W_q ; K = x @ W_k ; V = x @ W_v
    I  = Q @ K.T  (causal masked, NO 1/sqrt(d) scaling)
    F  = softmax(I) @ V
    returns (F, stack([K, V]))

Key algebraic fold (host side): xp is not an output, so
    Q = x_b @ (lin_w.T @ W_q) + (lin_b @ W_q) = x_b @ Wq' + bq
and likewise for K, V — the linear stage disappears from the device
entirely (Wq'/Wk'/Wv' and the bias vectors are precomputed in float64
on the host).

Device layout (hardcoded for S=2048, D=H=1024, B=8, batch-parallel on 8 cores):
  - host passes xT = x_b.T [D, S]; projections contract over d directly:
    Q^T/K^T [h, s] use Wq' chunks as stationary, xT as moving; V [s, h]
    uses xT chunks as stationary, Wv' as moving.
  - Q^T spills to DRAM, streamed back per 128-query chunk; K^T f32 and
    V bf16 stay resident in SBUF.
  - scores = Q^T.T @ K^T land [q, k] in PSUM; row softmax = free-dim
    reduce_max + ScalarE exp (per-partition -max bias, fused row-sum via
    accum_out); P~ cast bf16, transposed 128x128 on TensorE, then
    P~^T @ V accumulates F; 1/rowsum applied on the way out.
  - matmul dtypes: float32r (fp32 storage, ~1.5e-4 matmul rel err,
    bf16-rate at N=512) for the logit-sensitive chain; bf16 for P@V.
  - attention runs q-chunks DESCENDING with a one-chunk software pipeline
    (two-chunk for the small tail) so P@V fills the PE during softmax.
Measured: relF ~2.8e-3, relK/V ~2e-4 vs the fp32 reference.
"""

import sys

sys.path.insert(0, "/opt/trn_rl_repo")

import numpy as np

P = 128
S = 2048  # sequence length
D = 1024  # input size
H = 1024  # hidden size
FT = 512  # free-dim tile (one PSUM bank of fp32)
NB = 8  # batch == number of cores
DC = D // P  # 8 contraction chunks
HC = H // P
ST = S // FT  # 4 s-tiles
QC = S // P  # 16 query chunks
NEG = -1.0e30

_cache = {}


def _build():
    import concourse.mybir as mybir
    import concourse.tile as tile
    from concourse import bacc
    from concourse.masks import make_identity

    f32 = mybir.dt.float32
    f32r = mybir.dt.float32r
    bf16 = mybir.dt.bfloat16
    EXP = mybir.ActivationFunctionType.Exp
    AX = mybir.AxisListType.X

    nc = bacc.Bacc(None, target_bir_lowering=False)

    # all inputs pre-swizzled on the host so every DMA is long-contiguous
    # per SBUF partition (512B-run chunked loads were DMA-descriptor-bound)
    xT_d = nc.declare_dram_parameter("xT", [P, DC, S], f32r, isOutput=False)
    xv_d = nc.declare_dram_parameter("xv", [P, QC, DC, P], f32r, isOutput=False)
    wq_d = nc.declare_dram_parameter("Wq", [P, DC, H], f32r, isOutput=False)
    wk_d = nc.declare_dram_parameter("Wk", [P, HC, DC, P], f32r, isOutput=False)
    wv_d = nc.declare_dram_parameter("Wv", [P, DC, H], f32r, isOutput=False)
    bq_d = nc.declare_dram_parameter("bq", [P, HC], f32, isOutput=False)
    bk_d = nc.declare_dram_parameter("bk", [P, HC], f32, isOutput=False)
    bv_d = nc.declare_dram_parameter("bv", [P, H], f32, isOutput=False)
    mask_d = nc.declare_dram_parameter("masks", [4, P, FT], f32, isOutput=False)
    F_d = nc.declare_dram_parameter("F_out", [S, H], f32, isOutput=True)
    KT_d = nc.declare_dram_parameter("KT_out", [H, S], f32r, isOutput=True)
    V_d = nc.declare_dram_parameter("V_out", [S, H], f32r, isOutput=True)
    qt_spill = nc.dram_tensor("QT_spill", [H, S], f32r)

    with tile.TileContext(nc) as tc:
        qtp = tc.alloc_tile_pool(name="qtp", bufs=2)
        biasp = tc.alloc_tile_pool(name="biasp", bufs=1)
        stg = tc.alloc_tile_pool(name="stg", bufs=3)
        xtp = tc.alloc_tile_pool(name="xtp", bufs=1)
        wchp = tc.alloc_tile_pool(name="wchp", bufs=4)
        psmm = tc.alloc_tile_pool(name="psmm", bufs=8, space="PSUM")

        xt_sb = xtp.tile([P, DC, S], f32r, tag="xt", name="xt")
        bqk_sb = biasp.tile([P, 2, HC], f32, tag="bqk", name="bqk")
        bq_sb = bqk_sb[:, 0]
        bk_sb = bqk_sb[:, 1]
        bv_sb = biasp.tile([P, H], f32, tag="bv", name="bv")
        nc.sync.dma_start(bq_sb[:], bq_d.ap())
        nc.sync.dma_start(bk_sb[:], bk_d.ap())
        nc.sync.dma_start(bv_sb[:], bv_d.ap())
        wvp = tc.alloc_tile_pool(name="wvp", bufs=1, side="right")
        wv_sb = wvp.tile([P, DC, H], f32r, tag="wv", name="wv")
        wqp = tc.alloc_tile_pool(name="wqp", bufs=1)
        wq_sb = wqp.tile([P, DC, H], f32r, tag="wq", name="wq")
        # issue order = need order: xt st0, then Wq', then the rest of xt,
        # then Wv' (used ~100us later)
        for dc in range(DC):
            nc.sync.dma_start(xt_sb[:, dc, 0:FT], xT_d.ap()[:, dc, 0:FT])
        for dc in range(DC):
            nc.sync.dma_start(wq_sb[:, dc, :], wq_d.ap()[:, dc, :])
        for st in range(1, ST):
            for dc in range(DC):
                nc.sync.dma_start(
                    xt_sb[:, dc, st * FT : (st + 1) * FT],
                    xT_d.ap()[:, dc, st * FT : (st + 1) * FT],
                )

        # ---- Q^T -> DRAM spill; st-outer + resident Wq' for early start ----
        for st in range(ST):
            for hc in range(HC):
                pt = psmm.tile([P, FT], f32, tag="mm", name="mm")
                for dc in range(DC):
                    nc.tensor.matmul(
                        pt[:],
                        wq_sb[:, dc, hc * P : (hc + 1) * P],
                        xt_sb[:, dc, st * FT : (st + 1) * FT],
                        start=(dc == 0),
                        stop=(dc == DC - 1),
                    )
                qstg = stg.tile([P, FT], f32r, tag="stg", name="stg")
                nc.vector.tensor_add(
                    qstg[:], pt[:], bq_sb[:, hc : hc + 1].to_broadcast((P, FT))
                )
                nc.sync.dma_start(
                    qt_spill.ap()[hc * P : (hc + 1) * P, st * FT : (st + 1) * FT],
                    qstg[:],
                )
        wqp.release()
        for ec in range(DC):
            nc.sync.dma_start(wv_sb[:, ec, :], wv_d.ap()[:, ec, :])

        # prefetch the first two attention q-chunks' Q^T columns now
        qt_tiles = {}

        def load_qt(qi):
            qt = qtp.tile([P, HC, P], f32r, tag="qt", name="qt")
            for hc in range(HC):
                nc.sync.dma_start(
                    qt[:, hc, :],
                    qt_spill.ap()[hc * P : (hc + 1) * P, qi * P : (qi + 1) * P],
                )
            qt_tiles[qi] = qt

        load_qt(QC - 1)
        load_qt(QC - 2)

        # ---- K^T resident + K cache out (streamed Wk' chunks) ----
        ktp = tc.alloc_tile_pool(name="ktp", bufs=1, side="right")
        kt_sb = ktp.tile([P, HC, S], f32r, tag="kt", name="kt")
        wk_tiles = {}

        def load_wch(hc):
            wch = wchp.tile([P, DC, P], f32r, tag="wch", name="wch")
            nc.sync.dma_start(wch[:], wk_d.ap()[:, hc])
            wk_tiles[hc] = wch

        load_wch(0)
        load_wch(1)
        load_wch(2)
        for hc in range(HC):
            if hc + 3 < HC:
                load_wch(hc + 3)
            wch = wk_tiles.pop(hc)
            pts = [psmm.tile([P, FT], f32, tag="mm", name="mm") for _ in range(ST)]
            for ec in range(DC):
                for st in range(ST):
                    nc.tensor.matmul(
                        pts[st][:],
                        wch[:, ec, :],
                        xt_sb[:, ec, st * FT : (st + 1) * FT],
                        start=(ec == 0),
                        stop=(ec == DC - 1),
                    )
            for st in range(ST):
                nc.vector.tensor_add(
                    kt_sb[:, hc, st * FT : (st + 1) * FT],
                    pts[st][:],
                    bk_sb[:, hc : hc + 1].to_broadcast((P, FT)),
                )
                nc.sync.dma_start(
                    KT_d.ap()[hc * P : (hc + 1) * P, st * FT : (st + 1) * FT],
                    kt_sb[:, hc, st * FT : (st + 1) * FT],
                )

        # ---- V natural + cache out + bf16 copy ----
        # xT is streamed back from DRAM here so the big resident xt tile can
        # be released before the V stage (SBUF headroom for kt/v_bf).
        wchp.release()
        xtp.release()
        vxtp = tc.alloc_tile_pool(name="vxtp", bufs=6)
        vbfp = tc.alloc_tile_pool(name="vbfp", bufs=1, side="right")
        v_bf = vbfp.tile([P, QC, H], bf16, tag="vbf", name="vbf")
        vxt_tiles = {}

        def load_vxt(sc):
            vxt = vxtp.tile([P, DC, P], f32r, tag="vxt", name="vxt")
            nc.sync.dma_start(vxt[:], xv_d.ap()[:, sc])
            vxt_tiles[sc] = vxt

        for sc in range(5):
            load_vxt(sc)
        for sc in range(QC):
            if sc + 5 < QC:
                load_vxt(sc + 5)
            vxt = vxt_tiles.pop(sc)
            pts = [psmm.tile([P, FT], f32, tag="mm", name="mm") for _ in range(2)]
            for ec in range(DC):
                for ht in range(2):
                    nc.tensor.matmul(
                        pts[ht][:],
                        vxt[:, ec, :],
                        wv_sb[:, ec, ht * FT : (ht + 1) * FT],
                        start=(ec == 0),
                        stop=(ec == DC - 1),
                    )
            for ht in range(2):
                vstg = stg.tile([P, FT], f32r, tag="stg", name="stg")
                nc.vector.tensor_add(
                    vstg[:], pts[ht][:], bv_sb[:, ht * FT : (ht + 1) * FT]
                )
                nc.sync.dma_start(
                    V_d.ap()[sc * P : (sc + 1) * P, ht * FT : (ht + 1) * FT],
                    vstg[:],
                )
                nc.scalar.copy(v_bf[:, sc, ht * FT : (ht + 1) * FT], vstg[:])

        # ---- attention, one 128-query chunk at a time, DESCENDING ----
        vxtp.release()
        stg.release()
        psmm.release()
        with (
            tc.tile_pool(name="small", bufs=1) as small,
            tc.tile_pool(name="pbfp", bufs=2) as pbfp,
            tc.tile_pool(name="ptp", bufs=2) as ptp,
            tc.tile_pool(name="fp", bufs=2) as fp,
            tc.tile_pool(name="smp", bufs=3) as smp,
            tc.tile_pool(name="psS", bufs=6, space="PSUM") as psS,
            tc.tile_pool(name="psF", bufs=2, space="PSUM") as psF,
        ):
            mask_sb = small.tile([P, 4, FT], f32, tag="mask", name="mask")
            for v in range(4):
                nc.sync.dma_start(mask_sb[:, v, :], mask_d.ap()[v])
            ident = small.tile([P, P], bf16, tag="ident", name="ident")
            make_identity(nc, ident[:])

            def qk_block(qi):
                n_kt = qi // 4 + 1
                if qi - 2 >= 0:
                    load_qt(qi - 2)
                qt = qt_tiles.pop(qi)
                sts = [
                    psS.tile([P, FT], f32, tag="S", name="S") for _ in range(n_kt)
                ]
                for kt in range(n_kt):
                    for hc in range(HC):
                        nc.tensor.matmul(
                            sts[kt][:],
                            qt[:, hc, :],
                            kt_sb[:, hc, kt * FT : (kt + 1) * FT],
                            start=(hc == 0),
                            stop=(hc == HC - 1),
                        )
                return sts

            def softmax_block(qi, sts):
                n_kt = len(sts)
                v = qi % 4
                nc.vector.tensor_add(sts[-1][:], sts[-1][:], mask_sb[:, v, :])
                sm = smp.tile([P, 16], f32, tag="sm", name="sm")
                for kt in range(n_kt):
                    nc.vector.reduce_max(sm[:, kt : kt + 1], sts[kt][:], axis=AX)
                negm = sm[:, 8:9]
                nc.vector.reduce_max(negm, sm[:, :n_kt], axis=AX, negate=True)
                p_bf = pbfp.tile([P, S], bf16, tag="pbf", name="pbf")
                for kt in range(n_kt):
                    nc.scalar.activation(
                        p_bf[:, kt * FT : (kt + 1) * FT],
                        sts[kt][:],
                        EXP,
                        bias=negm,
                        accum_out=sm[:, 4 + kt : 5 + kt],
                    )
                recip = sm[:, 10:11]
                if n_kt > 1:
                    nc.vector.reduce_sum(sm[:, 9:10], sm[:, 4 : 4 + n_kt], axis=AX)
                    nc.vector.reciprocal(recip, sm[:, 9:10])
                else:
                    nc.vector.reciprocal(recip, sm[:, 4:5])
                ptb = ptp.tile([P, QC, P], bf16, tag="pt", name="pt")
                for kc in range(qi + 1):
                    tp = psS.tile([P, P], bf16, tag="S", name="S_tp")
                    nc.tensor.transpose(
                        tp[:], p_bf[:, kc * P : (kc + 1) * P], ident[:]
                    )
                    nc.vector.tensor_copy(ptb[:, kc, :], tp[:])
                return ptb, recip

            def pv_block(qi, ptb, recip):
                fts = [psF.tile([P, FT], f32, tag="F", name="F") for _ in range(2)]
                for kc in range(qi + 1):
                    for ht in range(2):
                        nc.tensor.matmul(
                            fts[ht][:],
                            ptb[:, kc, :],
                            v_bf[:, kc, ht * FT : (ht + 1) * FT],
                            start=(kc == 0),
                            stop=(kc == qi),
                        )
                fsb = fp.tile([P, H], f32, tag="fsb", name="fsb")
                for ht in range(2):
                    nc.vector.tensor_mul(
                        fsb[:, ht * FT : (ht + 1) * FT],
                        fts[ht][:],
                        recip.to_broadcast((P, FT)),
                    )
                nc.sync.dma_start(F_d.ap()[qi * P : (qi + 1) * P, :], fsb[:])

            pending = None
            for qi in range(QC - 1, 5, -1):
                sts = qk_block(qi)
                ptb, recip = softmax_block(qi, sts)
                if pending is not None:
                    pv_block(*pending)
                pending = (qi, ptb, recip)
            for a in (5, 3, 1):
                b = a - 1
                sts_a = qk_block(a)
                sts_b = qk_block(b)
                ptb_a, recip_a = softmax_block(a, sts_a)
                if pending is not None:
                    pv_block(*pending)
                ptb_b, recip_b = softmax_block(b, sts_b)
                pv_block(a, ptb_a, recip_a)
                pending = (b, ptb_b, recip_b)
            pv_block(*pending)
        vbfp.release()
        ktp.release()
        wvp.release()
        biasp.release()
        qtp.release()

    nc.compile()
    return nc


def _get_nc():
    if "nc" not in _cache:
        _cache["nc"] = _build()
    return _cache["nc"]


def _masks():
    m = np.full((4, P, FT), NEG, dtype=np.float32)
    j = np.arange(FT)[None, :]
    p = np.arange(P)[:, None]
    for v in range(4):
        m[v][j <= p + P * v] = 0.0
    return m


_last_in_maps = None


def kernel(x_batch, lin_w, lin_b, W_q, W_k, W_v):
    from concourse.bass_utils import run_bass_kernel_spmd

    nc = _get_nc()
    x_batch = np.asarray(x_batch, dtype=np.float32)
    lwT64 = np.asarray(lin_w, dtype=np.float64).T
    lb64 = np.asarray(lin_b, dtype=np.float64)
    cw = {}
    for nm, w in (("q", W_q), ("k", W_k), ("v", W_v)):
        w64 = np.asarray(w, dtype=np.float64)
        cw["W" + nm] = np.ascontiguousarray((lwT64 @ w64).astype(np.float32))
        cw["b" + nm] = (lb64 @ w64).astype(np.float32)
    bq = np.ascontiguousarray(cw["bq"].reshape(HC, P).T)  # [P, HC]
    bk = np.ascontiguousarray(cw["bk"].reshape(HC, P).T)
    bv = np.ascontiguousarray(np.tile(cw["bv"][None, :], (P, 1)))  # [P, H]
    masks = _masks()

    def sw_dPH(w):  # [D, H] -> [P, DC, H]
        return np.ascontiguousarray(w.reshape(DC, P, H).transpose(1, 0, 2))

    def sw_chunked(w):  # [D, H] -> [P, HC, DC, P]
        return np.ascontiguousarray(
            w.reshape(DC, P, HC, P).transpose(1, 2, 0, 3)
        )

    wq_sw = sw_dPH(cw["Wq"])
    wk_sw = sw_chunked(cw["Wk"])
    wv_sw = sw_dPH(cw["Wv"])
    in_maps = []
    for c in range(NB):
        xb = x_batch[c]  # [S, D]
        xT_sw = np.ascontiguousarray(
            xb.T.reshape(DC, P, S).transpose(1, 0, 2)
        )  # [P, DC, S]
        xv_sw = np.ascontiguousarray(
            xb.reshape(QC, P, DC, P).transpose(3, 0, 2, 1)
        )  # [P, QC, DC, P] : xv[p, sc, dc, j] = x[sc*128+j, dc*128+p]
        in_maps.append(
            {
                "xT": xT_sw,
                "xv": xv_sw,
                "Wq": wq_sw,
                "Wk": wk_sw,
                "Wv": wv_sw,
                "bq": bq,
                "bk": bk,
                "bv": bv,
                "masks": masks,
            }
        )
    global _last_in_maps
    _last_in_maps = in_maps
    res = run_bass_kernel_spmd(nc, in_maps, core_ids=list(range(NB)))
    F = np.stack([res.results[c]["F_out"] for c in range(NB)])
    K = np.stack([np.ascontiguousarray(res.results[c]["KT_out"].T) for c in range(NB)])
    V = np.stack([res.results[c]["V_out"] for c in range(NB)])
    cache = np.stack([K, V])
    return (F, cache)


# revision 17
# speedup vs baseline: 1.4555x; 1.0295x over previous
"""TRN2 Bass kernel for nn_Attention_1709396984084.

Reference computation (per batch element b, 8 of them -> one NeuronCore each):
    x  = x_b @ lin_w.T + lin_b          # [S, D]
    Q  = x @ W_q ; K = x @ W_k ; V = x @ W_v
    I  = Q @ K.T  (causal masked, NO 1/sqrt(d) scaling)
    F  = softmax(I) @ V
    returns (F, stack([K, V]))

Key algebraic fold (host side): xp is not an output, so
    Q = x_b @ (lin_w.T @ W_q) + (lin_b @ W_q) = x_b @ Wq' + bq
and likewise for K, V — the linear stage disappears from the device
entirely (Wq'/Wk'/Wv' and the bias vectors are precomputed in float64
on the host).

Device layout (hardcoded for S=2048, D=H=1024, B=8, batch-parallel on 8 cores):
  - host passes xT = x_b.T [D, S]; projections contract over d directly:
    Q^T/K^T [h, s] use Wq' chunks as stationary, xT as moving; V [s, h]
    uses xT chunks as stationary, Wv' as moving.
  - Q^T spills to DRAM, streamed back per 128-query chunk; K^T f32 and
    V bf16 stay resident in SBUF.
  - scores = Q^T.T @ K^T land [q, k] in PSUM; row softmax = free-dim
    reduce_max + ScalarE exp (per-partition -max bias, fused row-sum via
    accum_out); P~ cast bf16, transposed 128x128 on TensorE, then
    P~^T @ V accumulates F; 1/rowsum applied on the way out.
  - matmul dtypes: float32r (fp32 storage, ~1.5e-4 matmul rel err,
    bf16-rate at N=512) for the logit-sensitive chain; bf16 for P@V.
  - attention runs q-chunks DESCENDING with a one-chunk software pipeline
    (two-chunk for the small tail) so P@V fills the PE during softmax.
Measured: relF ~2.8e-3, relK/V ~2e-4 vs the fp32 reference.
"""

import sys

sys.path.insert(0, "/opt/trn_rl_repo")

import numpy as np

P = 128
S = 2048  # sequence length
D = 1024  # input size
H = 1024  # hidden size
FT = 512  # free-dim tile (one PSUM bank of fp32)
NB = 8  # batch == number of cores
DC = D // P  # 8 contraction chunks
HC = H // P
ST = S // FT  # 4 s-tiles
QC = S // P  # 16 query chunks
NEG = -1.0e30

_cache = {}


def _build():
    import concourse.mybir as mybir
    import concourse.tile as tile
    from concourse import bacc
    from concourse.masks import make_identity

    f32 = mybir.dt.float32
    f32r = mybir.dt.float32r
    bf16 = mybir.dt.bfloat16
    EXP = mybir.ActivationFunctionType.Exp
    AX = mybir.AxisListType.X

    nc = bacc.Bacc(None, target_bir_lowering=False)

    # all inputs pre-swizzled on the host so every DMA is long-contiguous
    # per SBUF partition (512B-run chunked loads were DMA-descriptor-bound)
    xT_d = nc.declare_dram_parameter("xT", [P, DC, S], f32r, isOutput=False)
    xv_d = nc.declare_dram_parameter("xv", [P, QC, DC, P], f32r, isOutput=False)
    wq_d = nc.declare_dram_parameter("Wq", [P, HC, DC, P], f32r, isOutput=False)
    wk_d = nc.declare_dram_parameter("Wk", [P, HC, DC, P], f32r, isOutput=False)
    wv_d = nc.declare_dram_parameter("Wv", [P, DC, H], f32r, isOutput=False)
    bq_d = nc.declare_dram_parameter("bq", [P, HC], f32, isOutput=False)
    bk_d = nc.declare_dram_parameter("bk", [P, HC], f32, isOutput=False)
    bv_d = nc.declare_dram_parameter("bv", [P, H], f32, isOutput=False)
    mask_d = nc.declare_dram_parameter("masks", [4, P, FT], f32, isOutput=False)
    F_d = nc.declare_dram_parameter("F_out", [S, H], f32, isOutput=True)
    KT_d = nc.declare_dram_parameter("KT_out", [H, S], f32r, isOutput=True)
    V_d = nc.declare_dram_parameter("V_out", [S, H], f32r, isOutput=True)
    qt_spill = nc.dram_tensor("QT_spill", [H, S], f32r)

    with tile.TileContext(nc) as tc:
        qtp = tc.alloc_tile_pool(name="qtp", bufs=2)
        biasp = tc.alloc_tile_pool(name="biasp", bufs=1)
        stg = tc.alloc_tile_pool(name="stg", bufs=3)
        wchp = tc.alloc_tile_pool(name="wchp", bufs=4)
        xtp = tc.alloc_tile_pool(name="xtp", bufs=1)
        psmm = tc.alloc_tile_pool(name="psmm", bufs=8, space="PSUM")

        xt_sb = xtp.tile([P, DC, S], f32r, tag="xt", name="xt")
        bqk_sb = biasp.tile([P, 2, HC], f32, tag="bqk", name="bqk")
        bq_sb = bqk_sb[:, 0]
        bk_sb = bqk_sb[:, 1]
        bv_sb = biasp.tile([P, H], f32, tag="bv", name="bv")
        nc.sync.dma_start(bq_sb[:], bq_d.ap())
        nc.sync.dma_start(bk_sb[:], bk_d.ap())
        nc.sync.dma_start(bv_sb[:], bv_d.ap())
        wvp = tc.alloc_tile_pool(name="wvp", bufs=1, side="right")
        wv_sb = wvp.tile([P, DC, H], f32r, tag="wv", name="wv")
        wqp = tc.alloc_tile_pool(name="wqp", bufs=1)
        wq_sb = wqp.tile([P, HC, DC, P], f32r, tag="wq", name="wq")
        # issue order = need order: xt st0, then Wq' (hc-major), then the
        # rest of xt; Wv' is deferred until the K stage
        for dc in range(DC):
            nc.sync.dma_start(xt_sb[:, dc, 0:FT], xT_d.ap()[:, dc, 0:FT])
        for hc in range(HC):
            nc.sync.dma_start(wq_sb[:, hc], wq_d.ap()[:, hc])
        for st in range(1, ST):
            for dc in range(DC):
                nc.sync.dma_start(
                    xt_sb[:, dc, st * FT : (st + 1) * FT],
                    xT_d.ap()[:, dc, st * FT : (st + 1) * FT],
                )

        # ---- Q^T -> DRAM spill; st-outer + resident Wq' for early start ----
        for st in range(ST):
            for hc in range(HC):
                pt = psmm.tile([P, FT], f32, tag="mm", name="mm")
                for dc in range(DC):
                    nc.tensor.matmul(
                        pt[:],
                        wq_sb[:, hc, dc, :],
                        xt_sb[:, dc, st * FT : (st + 1) * FT],
                        start=(dc == 0),
                        stop=(dc == DC - 1),
                    )
                qstg = stg.tile([P, FT], f32r, tag="stg", name="stg")
                nc.vector.tensor_add(
                    qstg[:], pt[:], bq_sb[:, hc : hc + 1].to_broadcast((P, FT))
                )
                nc.sync.dma_start(
                    qt_spill.ap()[hc * P : (hc + 1) * P, st * FT : (st + 1) * FT],
                    qstg[:],
                )
        wqp.release()

        # prefetch the first two attention q-chunks' Q^T columns now
        qt_tiles = {}

        def load_qt(qi):
            qt = qtp.tile([P, HC, P], f32r, tag="qt", name="qt")
            for hc in range(HC):
                nc.sync.dma_start(
                    qt[:, hc, :],
                    qt_spill.ap()[hc * P : (hc + 1) * P, qi * P : (qi + 1) * P],
                )
            qt_tiles[qi] = qt

        load_qt(QC - 1)
        load_qt(QC - 2)

        # ---- K^T resident + K cache out (streamed Wk' chunks) ----
        ktp = tc.alloc_tile_pool(name="ktp", bufs=1, side="right")
        kt_sb = ktp.tile([P, HC, S], f32r, tag="kt", name="kt")
        wk_tiles = {}

        def load_wch(hc):
            wch = wchp.tile([P, DC, P], f32r, tag="wch", name="wch")
            nc.sync.dma_start(wch[:], wk_d.ap()[:, hc])
            wk_tiles[hc] = wch

        load_wch(0)
        load_wch(1)
        load_wch(2)
        for ec in range(DC):
            nc.sync.dma_start(wv_sb[:, ec, :], wv_d.ap()[:, ec, :])
        for hc in range(HC):
            if hc + 3 < HC:
                load_wch(hc + 3)
            wch = wk_tiles.pop(hc)
            pts = [psmm.tile([P, FT], f32, tag="mm", name="mm") for _ in range(ST)]
            for ec in range(DC):
                for st in range(ST):
                    nc.tensor.matmul(
                        pts[st][:],
                        wch[:, ec, :],
                        xt_sb[:, ec, st * FT : (st + 1) * FT],
                        start=(ec == 0),
                        stop=(ec == DC - 1),
                    )
            for st in range(ST):
                nc.vector.tensor_add(
                    kt_sb[:, hc, st * FT : (st + 1) * FT],
                    pts[st][:],
                    bk_sb[:, hc : hc + 1].to_broadcast((P, FT)),
                )
                nc.sync.dma_start(
                    KT_d.ap()[hc * P : (hc + 1) * P, st * FT : (st + 1) * FT],
                    kt_sb[:, hc, st * FT : (st + 1) * FT],
                )

        # ---- V natural + cache out + bf16 copy ----
        # xT is streamed back from DRAM (swizzled copy) via the same chunk
        # pool the K weights used, so prefetch flows across the boundary.
        xtp.release()
        vbfp = tc.alloc_tile_pool(name="vbfp", bufs=1, side="right")
        v_bf = vbfp.tile([P, QC, H], bf16, tag="vbf", name="vbf")
        vxt_tiles = {}

        def load_vxt(sc):
            vxt = wchp.tile([P, DC, P], f32r, tag="wch", name="wch")
            nc.sync.dma_start(vxt[:], xv_d.ap()[:, sc])
            vxt_tiles[sc] = vxt

        # 4) attention constants early: masks + identity load during V
        small = tc.alloc_tile_pool(name="small", bufs=1, side="right")
        mask_sb = small.tile([P, 4, FT], f32, tag="mask", name="mask")
        for v in range(4):
            nc.sync.dma_start(mask_sb[:, v, :], mask_d.ap()[v])
        ident = small.tile([P, P], bf16, tag="ident", name="ident")
        make_identity(nc, ident[:])

        for sc in range(3):
            load_vxt(sc)
        for sc in range(QC):
            if sc + 3 < QC:
                load_vxt(sc + 3)
            vxt = vxt_tiles.pop(sc)
            pts = [psmm.tile([P, FT], f32, tag="mm", name="mm") for _ in range(2)]
            for ec in range(DC):
                for ht in range(2):
                    nc.tensor.matmul(
                        pts[ht][:],
                        vxt[:, ec, :],
                        wv_sb[:, ec, ht * FT : (ht + 1) * FT],
                        start=(ec == 0),
                        stop=(ec == DC - 1),
                    )
            for ht in range(2):
                vstg = stg.tile([P, FT], f32r, tag="stg", name="stg")
                nc.vector.tensor_add(
                    vstg[:], pts[ht][:], bv_sb[:, ht * FT : (ht + 1) * FT]
                )
                nc.sync.dma_start(
                    V_d.ap()[sc * P : (sc + 1) * P, ht * FT : (ht + 1) * FT],
                    vstg[:],
                )
                nc.scalar.copy(v_bf[:, sc, ht * FT : (ht + 1) * FT], vstg[:])

        # ---- attention, one 128-query chunk at a time, DESCENDING ----
        wchp.release()
        stg.release()
        psmm.release()
        with (
            tc.tile_pool(name="pbfp", bufs=2) as pbfp,
            tc.tile_pool(name="ptp", bufs=2) as ptp,
            tc.tile_pool(name="fp", bufs=2) as fp,
            tc.tile_pool(name="smp", bufs=3) as smp,
            tc.tile_pool(name="psS", bufs=6, space="PSUM") as psS,
            tc.tile_pool(name="psF", bufs=2, space="PSUM") as psF,
        ):

            def qk_block(qi):
                n_kt = qi // 4 + 1
                if qi - 2 >= 0:
                    load_qt(qi - 2)
                qt = qt_tiles.pop(qi)
                sts = [
                    psS.tile([P, FT], f32, tag="S", name="S") for _ in range(n_kt)
                ]
                for kt in range(n_kt):
                    for hc in range(HC):
                        nc.tensor.matmul(
                            sts[kt][:],
                            qt[:, hc, :],
                            kt_sb[:, hc, kt * FT : (kt + 1) * FT],
                            start=(hc == 0),
                            stop=(hc == HC - 1),
                        )
                return sts

            def softmax_block(qi, sts):
                n_kt = len(sts)
                v = qi % 4
                nc.vector.tensor_add(sts[-1][:], sts[-1][:], mask_sb[:, v, :])
                sm = smp.tile([P, 16], f32, tag="sm", name="sm")
                for kt in range(n_kt):
                    nc.vector.reduce_max(sm[:, kt : kt + 1], sts[kt][:], axis=AX)
                negm = sm[:, 8:9]
                nc.vector.reduce_max(negm, sm[:, :n_kt], axis=AX, negate=True)
                p_bf = pbfp.tile([P, S], bf16, tag="pbf", name="pbf")
                for kt in range(n_kt):
                    nc.scalar.activation(
                        p_bf[:, kt * FT : (kt + 1) * FT],
                        sts[kt][:],
                        EXP,
                        bias=negm,
                        accum_out=sm[:, 4 + kt : 5 + kt],
                    )
                recip = sm[:, 10:11]
                if n_kt > 1:
                    nc.vector.reduce_sum(sm[:, 9:10], sm[:, 4 : 4 + n_kt], axis=AX)
                    nc.vector.reciprocal(recip, sm[:, 9:10])
                else:
                    nc.vector.reciprocal(recip, sm[:, 4:5])
                ptb = ptp.tile([P, QC, P], bf16, tag="pt", name="pt")
                for kc in range(qi + 1):
                    tp = psS.tile([P, P], bf16, tag="S", name="S_tp")
                    nc.tensor.transpose(
                        tp[:], p_bf[:, kc * P : (kc + 1) * P], ident[:]
                    )
                    nc.vector.tensor_copy(ptb[:, kc, :], tp[:])
                return ptb, recip

            def pv_block(qi, ptb, recip):
                fts = [psF.tile([P, FT], f32, tag="F", name="F") for _ in range(2)]
                for kc in range(qi + 1):
                    for ht in range(2):
                        nc.tensor.matmul(
                            fts[ht][:],
                            ptb[:, kc, :],
                            v_bf[:, kc, ht * FT : (ht + 1) * FT],
                            start=(kc == 0),
                            stop=(kc == qi),
                        )
                fsb = fp.tile([P, H], f32, tag="fsb", name="fsb")
                for ht in range(2):
                    nc.vector.tensor_mul(
                        fsb[:, ht * FT : (ht + 1) * FT],
                        fts[ht][:],
                        recip.to_broadcast((P, FT)),
                    )
                nc.sync.dma_start(F_d.ap()[qi * P : (qi + 1) * P, :], fsb[:])

            pending = None
            for qi in range(QC - 1, 5, -1):
                sts = qk_block(qi)
                ptb, recip = softmax_block(qi, sts)
                if pending is not None:
                    pv_block(*pending)
                pending = (qi, ptb, recip)
            for a in (5, 3, 1):
                b = a - 1
                sts_a = qk_block(a)
                sts_b = qk_block(b)
                ptb_a, recip_a = softmax_block(a, sts_a)
                if pending is not None:
                    pv_block(*pending)
                ptb_b, recip_b = softmax_block(b, sts_b)
                pv_block(a, ptb_a, recip_a)
                pending = (b, ptb_b, recip_b)
            pv_block(*pending)
        small.release()
        vbfp.release()
        ktp.release()
        wvp.release()
        biasp.release()
        qtp.release()

    nc.compile()
    return nc


def _get_nc():
    if "nc" not in _cache:
        _cache["nc"] = _build()
    return _cache["nc"]


def _masks():
    m = np.full((4, P, FT), NEG, dtype=np.float32)
    j = np.arange(FT)[None, :]
    p = np.arange(P)[:, None]
    for v in range(4):
        m[v][j <= p + P * v] = 0.0
    return m


_last_in_maps = None


def kernel(x_batch, lin_w, lin_b, W_q, W_k, W_v):
    from concourse.bass_utils import run_bass_kernel_spmd

    nc = _get_nc()
    x_batch = np.asarray(x_batch, dtype=np.float32)
    lwT64 = np.asarray(lin_w, dtype=np.float64).T
    lb64 = np.asarray(lin_b, dtype=np.float64)
    cw = {}
    for nm, w in (("q", W_q), ("k", W_k), ("v", W_v)):
        w64 = np.asarray(w, dtype=np.float64)
        cw["W" + nm] = np.ascontiguousarray((lwT64 @ w64).astype(np.float32))
        cw["b" + nm] = (lb64 @ w64).astype(np.float32)
    bq = np.ascontiguousarray(cw["bq"].reshape(HC, P).T)  # [P, HC]
    bk = np.ascontiguousarray(cw["bk"].reshape(HC, P).T)
    bv = np.ascontiguousarray(np.tile(cw["bv"][None, :], (P, 1)))  # [P, H]
    masks = _masks()

    def sw_dPH(w):  # [D, H] -> [P, DC, H]
        return np.ascontiguousarray(w.reshape(DC, P, H).transpose(1, 0, 2))

    def sw_chunked(w):  # [D, H] -> [P, HC, DC, P]
        return np.ascontiguousarray(
            w.reshape(DC, P, HC, P).transpose(1, 2, 0, 3)
        )

    wq_sw = sw_chunked(cw["Wq"])
    wk_sw = sw_chunked(cw["Wk"])
    wv_sw = sw_dPH(cw["Wv"])
    in_maps = []
    for c in range(NB):
        xb = x_batch[c]  # [S, D]
        xT_sw = np.ascontiguousarray(
            xb.T.reshape(DC, P, S).transpose(1, 0, 2)
        )  # [P, DC, S]
        xv_sw = np.ascontiguousarray(
            xb.reshape(QC, P, DC, P).transpose(3, 0, 2, 1)
        )  # [P, QC, DC, P] : xv[p, sc, dc, j] = x[sc*128+j, dc*128+p]
        in_maps.append(
            {
                "xT": xT_sw,
                "xv": xv_sw,
                "Wq": wq_sw,
                "Wk": wk_sw,
                "Wv": wv_sw,
                "bq": bq,
                "bk": bk,
                "bv": bv,
                "masks": masks,
            }
        )
    global _last_in_maps
    _last_in_maps = in_maps
    res = run_bass_kernel_spmd(nc, in_maps, core_ids=list(range(NB)))
    F = np.stack([res.results[c]["F_out"] for c in range(NB)])
    K = np.stack([np.ascontiguousarray(res.results[c]["KT_out"].T) for c in range(NB)])
    V = np.stack([res.results[c]["V_out"] for c in range(NB)])
    cache = np.stack([K, V])
    return (F, cache)


# revision 20
# speedup vs baseline: 1.4800x; 1.0169x over previous
"""TRN2 Bass kernel for nn_Attention_1709396984084.

Reference computation (per batch element b, 8 of them -> one NeuronCore each):
    x  = x_b @ lin_w.T + lin_b          # [S, D]
    Q  = x @ W_q ; K = x @ W_k ; V = x @ W_v
    I  = Q @ K.T  (causal masked, NO 1/sqrt(d) scaling)
    F  = softmax(I) @ V
    returns (F, stack([K, V]))

Key algebraic fold (host side): xp is not an output, so
    Q = x_b @ (lin_w.T @ W_q) + (lin_b @ W_q) = x_b @ Wq' + bq
and likewise for K, V — the linear stage disappears from the device
entirely (Wq'/Wk'/Wv' and the bias vectors are precomputed in float64
on the host).

Device layout (hardcoded for S=2048, D=H=1024, B=8, batch-parallel on 8 cores):
  - host passes xT = x_b.T [D, S]; projections contract over d directly:
    Q^T/K^T [h, s] use Wq' chunks as stationary, xT as moving; V [s, h]
    uses xT chunks as stationary, Wv' as moving.
  - Q^T spills to DRAM, streamed back per 128-query chunk; K^T f32 and
    V bf16 stay resident in SBUF.
  - scores = Q^T.T @ K^T land [q, k] in PSUM; row softmax = free-dim
    reduce_max + ScalarE exp (per-partition -max bias, fused row-sum via
    accum_out); P~ cast bf16, transposed 128x128 on TensorE, then
    P~^T @ V accumulates F; 1/rowsum applied on the way out.
  - matmul dtypes: float32r (fp32 storage, ~1.5e-4 matmul rel err,
    bf16-rate at N=512) for the logit-sensitive chain; bf16 for P@V.
  - attention runs q-chunks DESCENDING with a one-chunk software pipeline
    (two-chunk for the small tail) so P@V fills the PE during softmax.
Measured: relF ~2.8e-3, relK/V ~2e-4 vs the fp32 reference.
"""

import sys

sys.path.insert(0, "/opt/trn_rl_repo")

import numpy as np

P = 128
S = 2048  # sequence length
D = 1024  # input size
H = 1024  # hidden size
FT = 512  # free-dim tile (one PSUM bank of fp32)
NB = 8  # batch == number of cores
DC = D // P  # 8 contraction chunks
HC = H // P
ST = S // FT  # 4 s-tiles
QC = S // P  # 16 query chunks
NEG = -1.0e30

_cache = {}


def _build():
    import concourse.mybir as mybir
    import concourse.tile as tile
    from concourse import bacc
    from concourse.masks import make_identity

    f32 = mybir.dt.float32
    f32r = mybir.dt.float32r
    bf16 = mybir.dt.bfloat16
    EXP = mybir.ActivationFunctionType.Exp
    AX = mybir.AxisListType.X

    nc = bacc.Bacc(None, target_bir_lowering=False)

    # all inputs pre-swizzled on the host so every DMA is long-contiguous
    # per SBUF partition (512B-run chunked loads were DMA-descriptor-bound)
    xT_d = nc.declare_dram_parameter("xT", [P, DC, S], f32r, isOutput=False)
    xv_d = nc.declare_dram_parameter("xv", [P, QC, DC, P], f32r, isOutput=False)
    wq_d = nc.declare_dram_parameter("Wq", [P, HC, DC, P], f32r, isOutput=False)
    wk_d = nc.declare_dram_parameter("Wk", [P, HC, DC, P], f32r, isOutput=False)
    wv_d = nc.declare_dram_parameter("Wv", [P, DC, H], f32r, isOutput=False)
    bq_d = nc.declare_dram_parameter("bq", [P, HC], f32, isOutput=False)
    bk_d = nc.declare_dram_parameter("bk", [P, HC], f32, isOutput=False)
    bv_d = nc.declare_dram_parameter("bv", [P, H], f32, isOutput=False)
    mask_d = nc.declare_dram_parameter("masks", [4, P, FT], f32, isOutput=False)
    F_d = nc.declare_dram_parameter("F_out", [S, H], f32, isOutput=True)
    KT_d = nc.declare_dram_parameter("KT_out", [H, S], f32r, isOutput=True)
    V_d = nc.declare_dram_parameter("V_out", [S, H], f32r, isOutput=True)
    qt_spill = nc.dram_tensor("QT_spill", [H, S], f32r)

    with tile.TileContext(nc) as tc:
        qtp = tc.alloc_tile_pool(name="qtp", bufs=2)
        biasp = tc.alloc_tile_pool(name="biasp", bufs=1)
        stg = tc.alloc_tile_pool(name="stg", bufs=3)
        wchp = tc.alloc_tile_pool(name="wchp", bufs=4)
        xtp = tc.alloc_tile_pool(name="xtp", bufs=1)
        psmm = tc.alloc_tile_pool(name="psmm", bufs=8, space="PSUM")

        xt_sb = xtp.tile([P, DC, S], f32r, tag="xt", name="xt")
        bqk_sb = biasp.tile([P, 2, HC], f32, tag="bqk", name="bqk")
        bq_sb = bqk_sb[:, 0]
        bk_sb = bqk_sb[:, 1]
        bv_sb = biasp.tile([P, H], f32, tag="bv", name="bv")

        wvp = tc.alloc_tile_pool(name="wvp", bufs=1, side="right")
        wv_sb = wvp.tile([P, DC, H], f32r, tag="wv", name="wv")
        wqp = tc.alloc_tile_pool(name="wqp", bufs=1)
        wq_sb = wqp.tile([P, HC, DC, P], f32r, tag="wq", name="wq")
        # issue order = need order: xt st0, then Wq' (hc-major), then the
        # rest of xt; Wv' is deferred until the K stage
        for dc in range(DC):
            nc.sync.dma_start(xt_sb[:, dc, 0:FT], xT_d.ap()[:, dc, 0:FT])
        for hc in range(HC):
            nc.sync.dma_start(wq_sb[:, hc], wq_d.ap()[:, hc])
        nc.sync.dma_start(bq_sb[:], bq_d.ap())
        nc.sync.dma_start(bk_sb[:], bk_d.ap())
        nc.sync.dma_start(bv_sb[:], bv_d.ap())
        for st in range(1, ST):
            for dc in range(DC):
                nc.sync.dma_start(
                    xt_sb[:, dc, st * FT : (st + 1) * FT],
                    xT_d.ap()[:, dc, st * FT : (st + 1) * FT],
                )

        # ---- Q^T -> DRAM spill; st-outer + resident Wq' for early start ----
        for st in range(ST):
            for hc in range(HC):
                pt = psmm.tile([P, FT], f32, tag="mm", name="mm")
                for dc in range(DC):
                    nc.tensor.matmul(
                        pt[:],
                        wq_sb[:, hc, dc, :],
                        xt_sb[:, dc, st * FT : (st + 1) * FT],
                        start=(dc == 0),
                        stop=(dc == DC - 1),
                    )
                qstg = stg.tile([P, FT], f32r, tag="stg", name="stg")
                nc.vector.tensor_add(
                    qstg[:], pt[:], bq_sb[:, hc : hc + 1].to_broadcast((P, FT))
                )
                nc.sync.dma_start(
                    qt_spill.ap()[hc * P : (hc + 1) * P, st * FT : (st + 1) * FT],
                    qstg[:],
                )
        wqp.release()

        # prefetch the first two attention q-chunks' Q^T columns now
        qt_tiles = {}

        def load_qt(qi):
            qt = qtp.tile([P, HC, P], f32r, tag="qt", name="qt")
            for hc in range(HC):
                nc.sync.dma_start(
                    qt[:, hc, :],
                    qt_spill.ap()[hc * P : (hc + 1) * P, qi * P : (qi + 1) * P],
                )
            qt_tiles[qi] = qt

        load_qt(QC - 1)
        load_qt(QC - 2)

        # ---- K^T resident + K cache out (streamed Wk' chunks) ----
        ktp = tc.alloc_tile_pool(name="ktp", bufs=1, side="right")
        kt_sb = ktp.tile([P, HC, S], f32r, tag="kt", name="kt")
        wk_tiles = {}

        def load_wch(hc):
            wch = wchp.tile([P, DC, P], f32r, tag="wch", name="wch")
            nc.sync.dma_start(wch[:], wk_d.ap()[:, hc])
            wk_tiles[hc] = wch

        load_wch(0)
        load_wch(1)
        load_wch(2)
        for ec in range(DC):
            nc.sync.dma_start(wv_sb[:, ec, :], wv_d.ap()[:, ec, :])
        for hc in range(HC):
            if hc + 3 < HC:
                load_wch(hc + 3)
            wch = wk_tiles.pop(hc)
            pts = [psmm.tile([P, FT], f32, tag="mm", name="mm") for _ in range(ST)]
            for ec in range(DC):
                for st in range(ST):
                    nc.tensor.matmul(
                        pts[st][:],
                        wch[:, ec, :],
                        xt_sb[:, ec, st * FT : (st + 1) * FT],
                        start=(ec == 0),
                        stop=(ec == DC - 1),
                    )
            for st in range(ST):
                nc.vector.tensor_add(
                    kt_sb[:, hc, st * FT : (st + 1) * FT],
                    pts[st][:],
                    bk_sb[:, hc : hc + 1].to_broadcast((P, FT)),
                )
                nc.sync.dma_start(
                    KT_d.ap()[hc * P : (hc + 1) * P, st * FT : (st + 1) * FT],
                    kt_sb[:, hc, st * FT : (st + 1) * FT],
                )

        # ---- V natural + cache out + bf16 copy ----
        # xT is streamed back from DRAM (swizzled copy) via the same chunk
        # pool the K weights used, so prefetch flows across the boundary.
        xtp.release()
        vbfp = tc.alloc_tile_pool(name="vbfp", bufs=1, side="right")
        v_bf = vbfp.tile([P, QC, H], bf16, tag="vbf", name="vbf")
        vxt_tiles = {}

        def load_vxt(sc):
            vxt = wchp.tile([P, DC, P], f32r, tag="wch", name="wch")
            nc.sync.dma_start(vxt[:], xv_d.ap()[:, sc])
            vxt_tiles[sc] = vxt

        # 4) attention constants early: masks + identity load during V
        small = tc.alloc_tile_pool(name="small", bufs=1, side="right")
        mask_sb = small.tile([P, 4, FT], f32, tag="mask", name="mask")
        for v in range(4):
            nc.sync.dma_start(mask_sb[:, v, :], mask_d.ap()[v])
        ident = small.tile([P, P], bf16, tag="ident", name="ident")
        make_identity(nc, ident[:])

        for sc in range(3):
            load_vxt(sc)
        for sc in range(QC):
            if sc + 3 < QC:
                load_vxt(sc + 3)
            vxt = vxt_tiles.pop(sc)
            pts = [psmm.tile([P, FT], f32, tag="mm", name="mm") for _ in range(2)]
            for ec in range(DC):
                for ht in range(2):
                    nc.tensor.matmul(
                        pts[ht][:],
                        vxt[:, ec, :],
                        wv_sb[:, ec, ht * FT : (ht + 1) * FT],
                        start=(ec == 0),
                        stop=(ec == DC - 1),
                    )
            for ht in range(2):
                vstg = stg.tile([P, FT], f32r, tag="stg", name="stg")
                nc.vector.tensor_add(
                    vstg[:], pts[ht][:], bv_sb[:, ht * FT : (ht + 1) * FT]
                )
                nc.sync.dma_start(
                    V_d.ap()[sc * P : (sc + 1) * P, ht * FT : (ht + 1) * FT],
                    vstg[:],
                )
                nc.scalar.copy(v_bf[:, sc, ht * FT : (ht + 1) * FT], vstg[:])

        # ---- attention, one 128-query chunk at a time, DESCENDING ----
        wchp.release()
        stg.release()
        psmm.release()
        with (
            tc.tile_pool(name="pbfp", bufs=2) as pbfp,
            tc.tile_pool(name="ptp", bufs=2) as ptp,
            tc.tile_pool(name="fp", bufs=2) as fp,
            tc.tile_pool(name="smp", bufs=3) as smp,
            tc.tile_pool(name="psS", bufs=6, space="PSUM") as psS,
            tc.tile_pool(name="psF", bufs=2, space="PSUM") as psF,
        ):

            def qk_block(qi):
                n_kt = qi // 4 + 1
                if qi - 2 >= 0:
                    load_qt(qi - 2)
                qt = qt_tiles.pop(qi)
                sts = [
                    psS.tile([P, FT], f32, tag="S", name="S") for _ in range(n_kt)
                ]
                for kt in range(n_kt):
                    for hc in range(HC):
                        nc.tensor.matmul(
                            sts[kt][:],
                            qt[:, hc, :],
                            kt_sb[:, hc, kt * FT : (kt + 1) * FT],
                            start=(hc == 0),
                            stop=(hc == HC - 1),
                        )
                return sts

            def softmax_block(qi, sts):
                n_kt = len(sts)
                v = qi % 4
                nc.vector.tensor_add(sts[-1][:], sts[-1][:], mask_sb[:, v, :])
                sm = smp.tile([P, 16], f32, tag="sm", name="sm")
                for kt in range(n_kt):
                    nc.vector.reduce_max(sm[:, kt : kt + 1], sts[kt][:], axis=AX)
                negm = sm[:, 8:9]
                nc.vector.reduce_max(negm, sm[:, :n_kt], axis=AX, negate=True)
                p_bf = pbfp.tile([P, S], bf16, tag="pbf", name="pbf")
                for kt in range(n_kt):
                    nc.scalar.activation(
                        p_bf[:, kt * FT : (kt + 1) * FT],
                        sts[kt][:],
                        EXP,
                        bias=negm,
                        accum_out=sm[:, 4 + kt : 5 + kt],
                    )
                recip = sm[:, 10:11]
                if n_kt > 1:
                    nc.vector.reduce_sum(sm[:, 9:10], sm[:, 4 : 4 + n_kt], axis=AX)
                    nc.vector.reciprocal(recip, sm[:, 9:10])
                else:
                    nc.vector.reciprocal(recip, sm[:, 4:5])
                ptb = ptp.tile([P, QC, P], bf16, tag="pt", name="pt")
                for kc in range(qi + 1):
                    tp = psS.tile([P, P], bf16, tag="S", name="S_tp")
                    nc.tensor.transpose(
                        tp[:], p_bf[:, kc * P : (kc + 1) * P], ident[:]
                    )
                    nc.vector.tensor_copy(ptb[:, kc, :], tp[:])
                return ptb, recip

            def pv_block(qi, ptb, recip):
                fts = [psF.tile([P, FT], f32, tag="F", name="F") for _ in range(2)]
                for kc in range(qi + 1):
                    for ht in range(2):
                        nc.tensor.matmul(
                            fts[ht][:],
                            ptb[:, kc, :],
                            v_bf[:, kc, ht * FT : (ht + 1) * FT],
                            start=(kc == 0),
                            stop=(kc == qi),
                        )
                fsb = fp.tile([P, H], f32, tag="fsb", name="fsb")
                for ht in range(2):
                    nc.vector.tensor_mul(
                        fsb[:, ht * FT : (ht + 1) * FT],
                        fts[ht][:],
                        recip.to_broadcast((P, FT)),
                    )
                nc.sync.dma_start(F_d.ap()[qi * P : (qi + 1) * P, :], fsb[:])

            pending = None
            for qi in range(QC - 1, 5, -1):
                sts = qk_block(qi)
                ptb, recip = softmax_block(qi, sts)
                if pending is not None:
                    pv_block(*pending)
                pending = (qi, ptb, recip)
            for a in (5, 3, 1):
                b = a - 1
                sts_a = qk_block(a)
                sts_b = qk_block(b)
                ptb_a, recip_a = softmax_block(a, sts_a)
                if pending is not None:
                    pv_block(*pending)
                ptb_b, recip_b = softmax_block(b, sts_b)
                pv_block(a, ptb_a, recip_a)
                pending = (b, ptb_b, recip_b)
            pv_block(*pending)
        small.release()
        vbfp.release()
        ktp.release()
        wvp.release()
        biasp.release()
        qtp.release()

    nc.compile()
    return nc


def _get_nc():
    if "nc" not in _cache:
        _cache["nc"] = _build()
    return _cache["nc"]


def _masks():
    m = np.full((4, P, FT), NEG, dtype=np.float32)
    j = np.arange(FT)[None, :]
    p = np.arange(P)[:, None]
    for v in range(4):
        m[v][j <= p + P * v] = 0.0
    return m


_last_in_maps = None


def kernel(x_batch, lin_w, lin_b, W_q, W_k, W_v):
    from concourse.bass_utils import run_bass_kernel_spmd

    nc = _get_nc()
    x_batch = np.asarray(x_batch, dtype=np.float32)
    lwT64 = np.asarray(lin_w, dtype=np.float64).T
    lb64 = np.asarray(lin_b, dtype=np.float64)
    cw = {}
    for nm, w in (("q", W_q), ("k", W_k), ("v", W_v)):
        w64 = np.asarray(w, dtype=np.float64)
        cw["W" + nm] = np.ascontiguousarray((lwT64 @ w64).astype(np.float32))
        cw["b" + nm] = (lb64 @ w64).astype(np.float32)
    bq = np.ascontiguousarray(cw["bq"].reshape(HC, P).T)  # [P, HC]
    bk = np.ascontiguousarray(cw["bk"].reshape(HC, P).T)
    bv = np.ascontiguousarray(np.tile(cw["bv"][None, :], (P, 1)))  # [P, H]
    masks = _masks()

    def sw_dPH(w):  # [D, H] -> [P, DC, H]
        return np.ascontiguousarray(w.reshape(DC, P, H).transpose(1, 0, 2))

    def sw_chunked(w):  # [D, H] -> [P, HC, DC, P]
        return np.ascontiguousarray(
            w.reshape(DC, P, HC, P).transpose(1, 2, 0, 3)
        )

    wq_sw = sw_chunked(cw["Wq"])
    wk_sw = sw_chunked(cw["Wk"])
    wv_sw = sw_dPH(cw["Wv"])
    in_maps = []
    for c in range(NB):
        xb = x_batch[c]  # [S, D]
        xT_sw = np.ascontiguousarray(
            xb.T.reshape(DC, P, S).transpose(1, 0, 2)
        )  # [P, DC, S]
        xv_sw = np.ascontiguousarray(
            xb.reshape(QC, P, DC, P).transpose(3, 0, 2, 1)
        )  # [P, QC, DC, P] : xv[p, sc, dc, j] = x[sc*128+j, dc*128+p]
        in_maps.append(
            {
                "xT": xT_sw,
                "xv": xv_sw,
                "Wq": wq_sw,
                "Wk": wk_sw,
                "Wv": wv_sw,
                "bq": bq,
                "bk": bk,
                "bv": bv,
                "masks": masks,
            }
        )
    global _last_in_maps
    _last_in_maps = in_maps
    res = run_bass_kernel_spmd(nc, in_maps, core_ids=list(range(NB)))
    F = np.stack([res.results[c]["F_out"] for c in range(NB)])
    K = np.stack([np.ascontiguousarray(res.results[c]["KT_out"].T) for c in range(NB)])
    V = np.stack([res.results[c]["V_out"] for c in range(NB)])
    cache = np.stack([K, V])
    return (F, cache)


# revision 21
# speedup vs baseline: 1.4820x; 1.0013x over previous
"""TRN2 Bass kernel for nn_Attention_1709396984084.

Reference computation (per batch element b, 8 of them -> one NeuronCore each):
    x  = x_b @ lin_w.T + lin_b          # [S, D]
    Q  = x @ W_q ; K = x @ W_k ; V = x @ W_v
    I  = Q @ K.T  (causal masked, NO 1/sqrt(d) scaling)
    F  = softmax(I) @ V
    returns (F, stack([K, V]))

Key algebraic fold (host side): xp is not an output, so
    Q = x_b @ (lin_w.T @ W_q) + (lin_b @ W_q) = x_b @ Wq' + bq
and likewise for K, V — the linear stage disappears from the device
entirely (Wq'/Wk'/Wv' and the bias vectors are precomputed in float64
on the host).

Device layout (hardcoded for S=2048, D=H=1024, B=8, batch-parallel on 8 cores):
  - host passes xT = x_b.T [D, S]; projections contract over d directly:
    Q^T/K^T [h, s] use Wq' chunks as stationary, xT as moving; V [s, h]
    uses xT chunks as stationary, Wv' as moving.
  - Q^T spills to DRAM, streamed back per 128-query chunk; K^T f32 and
    V bf16 stay resident in SBUF.
  - scores = Q^T.T @ K^T land [q, k] in PSUM; row softmax = free-dim
    reduce_max + ScalarE exp (per-partition -max bias, fused row-sum via
    accum_out); P~ cast bf16, transposed 128x128 on TensorE, then
    P~^T @ V accumulates F; 1/rowsum applied on the way out.
  - matmul dtypes: float32r (fp32 storage, ~1.5e-4 matmul rel err,
    bf16-rate at N=512) for the logit-sensitive chain; bf16 for P@V.
  - attention runs q-chunks DESCENDING with a one-chunk software pipeline
    (two-chunk for the small tail) so P@V fills the PE during softmax.
Measured: relF ~2.8e-3, relK/V ~2e-4 vs the fp32 reference.
"""

import sys

sys.path.insert(0, "/opt/trn_rl_repo")

import numpy as np

P = 128
S = 2048  # sequence length
D = 1024  # input size
H = 1024  # hidden size
FT = 512  # free-dim tile (one PSUM bank of fp32)
NB = 8  # batch == number of cores
DC = D // P  # 8 contraction chunks
HC = H // P
ST = S // FT  # 4 s-tiles
QC = S // P  # 16 query chunks
NEG = -1.0e30

_cache = {}


def _build():
    import concourse.mybir as mybir
    import concourse.tile as tile
    from concourse import bacc
    from concourse.masks import make_identity

    f32 = mybir.dt.float32
    f32r = mybir.dt.float32r
    bf16 = mybir.dt.bfloat16
    EXP = mybir.ActivationFunctionType.Exp
    AX = mybir.AxisListType.X

    nc = bacc.Bacc(None, target_bir_lowering=False)

    # all inputs pre-swizzled on the host so every DMA is long-contiguous
    # per SBUF partition (512B-run chunked loads were DMA-descriptor-bound)
    xT_d = nc.declare_dram_parameter("xT", [P, DC, S], f32r, isOutput=False)
    xv_d = nc.declare_dram_parameter("xv", [P, QC, DC, P], f32r, isOutput=False)
    wq_d = nc.declare_dram_parameter("Wq", [P, HC, DC, P], f32r, isOutput=False)
    wk_d = nc.declare_dram_parameter("Wk", [P, HC, DC, P], f32r, isOutput=False)
    wv_d = nc.declare_dram_parameter("Wv", [P, DC, H], f32r, isOutput=False)
    bq_d = nc.declare_dram_parameter("bq", [P, HC], f32, isOutput=False)
    bk_d = nc.declare_dram_parameter("bk", [P, HC], f32, isOutput=False)
    bv_d = nc.declare_dram_parameter("bv", [P, H], f32, isOutput=False)
    mask_d = nc.declare_dram_parameter("masks", [4, P, FT], f32, isOutput=False)
    F_d = nc.declare_dram_parameter("F_out", [S, H], f32, isOutput=True)
    KT_d = nc.declare_dram_parameter("KT_out", [H, S], f32r, isOutput=True)
    V_d = nc.declare_dram_parameter("V_out", [S, H], f32r, isOutput=True)
    qt_spill = nc.dram_tensor("QT_spill", [H, S], f32r)

    with tile.TileContext(nc) as tc:
        qtp = tc.alloc_tile_pool(name="qtp", bufs=2)
        biasp = tc.alloc_tile_pool(name="biasp", bufs=1)
        stg = tc.alloc_tile_pool(name="stg", bufs=3)
        wchp = tc.alloc_tile_pool(name="wchp", bufs=4)
        xtp = tc.alloc_tile_pool(name="xtp", bufs=1)
        psmm = tc.alloc_tile_pool(name="psmm", bufs=8, space="PSUM")

        xt_sb = xtp.tile([P, DC, S], f32r, tag="xt", name="xt")
        bqk_sb = biasp.tile([P, 2, HC], f32, tag="bqk", name="bqk")
        bq_sb = bqk_sb[:, 0]
        bk_sb = bqk_sb[:, 1]
        bv_sb = biasp.tile([P, H], f32, tag="bv", name="bv")

        wvp = tc.alloc_tile_pool(name="wvp", bufs=1, side="right")
        wv_sb = wvp.tile([P, DC, H], f32r, tag="wv", name="wv")
        wqp = tc.alloc_tile_pool(name="wqp", bufs=1)
        wq_sb = wqp.tile([P, HC, DC, P], f32r, tag="wq", name="wq")
        # issue order = need order: xt st0, then Wq' (hc-major), then the
        # rest of xt; Wv' is deferred until the K stage
        for dc in range(DC):
            nc.sync.dma_start(xt_sb[:, dc, 0:FT], xT_d.ap()[:, dc, 0:FT])
        for hc in range(HC):
            nc.sync.dma_start(wq_sb[:, hc], wq_d.ap()[:, hc])
        nc.sync.dma_start(bq_sb[:], bq_d.ap())
        nc.sync.dma_start(bk_sb[:], bk_d.ap())
        nc.sync.dma_start(bv_sb[:], bv_d.ap())
        for st in range(1, ST):
            for dc in range(DC):
                nc.sync.dma_start(
                    xt_sb[:, dc, st * FT : (st + 1) * FT],
                    xT_d.ap()[:, dc, st * FT : (st + 1) * FT],
                )

        # ---- Q^T -> DRAM spill; st-outer + resident Wq' for early start ----
        for st in range(ST):
            for hc in range(HC):
                pt = psmm.tile([P, FT], f32, tag="mm", name="mm")
                for dc in range(DC):
                    nc.tensor.matmul(
                        pt[:],
                        wq_sb[:, hc, dc, :],
                        xt_sb[:, dc, st * FT : (st + 1) * FT],
                        start=(dc == 0),
                        stop=(dc == DC - 1),
                    )
                qstg = stg.tile([P, FT], f32r, tag="stg", name="stg")
                nc.vector.tensor_add(
                    qstg[:], pt[:], bq_sb[:, hc : hc + 1].to_broadcast((P, FT))
                )
                nc.sync.dma_start(
                    qt_spill.ap()[hc * P : (hc + 1) * P, st * FT : (st + 1) * FT],
                    qstg[:],
                )
        wqp.release()

        # prefetch the first two attention q-chunks' Q^T columns now
        qt_tiles = {}

        def load_qt(qi):
            qt = qtp.tile([P, HC, P], f32r, tag="qt", name="qt")
            for hc in range(HC):
                nc.sync.dma_start(
                    qt[:, hc, :],
                    qt_spill.ap()[hc * P : (hc + 1) * P, qi * P : (qi + 1) * P],
                )
            qt_tiles[qi] = qt

        load_qt(QC - 1)
        load_qt(QC - 2)

        # ---- K^T resident + K cache out (streamed Wk' chunks) ----
        ktp = tc.alloc_tile_pool(name="ktp", bufs=1, side="right")
        kt_sb = ktp.tile([P, HC, S], f32r, tag="kt", name="kt")
        wk_tiles = {}

        def load_wch(hc):
            wch = wchp.tile([P, DC, P], f32r, tag="wch", name="wch")
            nc.sync.dma_start(wch[:], wk_d.ap()[:, hc])
            wk_tiles[hc] = wch

        load_wch(0)
        load_wch(1)
        load_wch(2)
        for ec in range(DC):
            nc.sync.dma_start(wv_sb[:, ec, :], wv_d.ap()[:, ec, :])
        for hc in range(HC):
            if hc + 3 < HC:
                load_wch(hc + 3)
            wch = wk_tiles.pop(hc)
            pts = [psmm.tile([P, FT], f32, tag="mm", name="mm") for _ in range(ST)]
            for ec in range(DC):
                for st in range(ST):
                    nc.tensor.matmul(
                        pts[st][:],
                        wch[:, ec, :],
                        xt_sb[:, ec, st * FT : (st + 1) * FT],
                        start=(ec == 0),
                        stop=(ec == DC - 1),
                    )
            for st in range(ST):
                nc.vector.tensor_add(
                    kt_sb[:, hc, st * FT : (st + 1) * FT],
                    pts[st][:],
                    bk_sb[:, hc : hc + 1].to_broadcast((P, FT)),
                )
                nc.sync.dma_start(
                    KT_d.ap()[hc * P : (hc + 1) * P, st * FT : (st + 1) * FT],
                    kt_sb[:, hc, st * FT : (st + 1) * FT],
                )

        # ---- V natural + cache out + bf16 copy ----
        # xT is streamed back from DRAM (swizzled copy) via the same chunk
        # pool the K weights used, so prefetch flows across the boundary.
        xtp.release()
        vbfp = tc.alloc_tile_pool(name="vbfp", bufs=1, side="right")
        v_bf = vbfp.tile([P, QC, H], bf16, tag="vbf", name="vbf")
        vxt_tiles = {}

        def load_vxt(sc):
            vxt = wchp.tile([P, DC, P], f32r, tag="wch", name="wch")
            nc.sync.dma_start(vxt[:], xv_d.ap()[:, sc])
            vxt_tiles[sc] = vxt

        # 4) attention constants early: masks + identity load during V
        small = tc.alloc_tile_pool(name="small", bufs=1, side="right")
        mask_sb = small.tile([P, 4, FT], f32, tag="mask", name="mask")
        for v in range(4):
            nc.sync.dma_start(mask_sb[:, v, :], mask_d.ap()[v])
        ident = small.tile([P, P], bf16, tag="ident", name="ident")
        make_identity(nc, ident[:])

        for sc in range(3):
            load_vxt(sc)
        for sc in range(QC):
            if sc + 3 < QC:
                load_vxt(sc + 3)
            vxt = vxt_tiles.pop(sc)
            pts = [psmm.tile([P, FT], f32, tag="mm", name="mm") for _ in range(2)]
            for ec in range(DC):
                for ht in range(2):
                    nc.tensor.matmul(
                        pts[ht][:],
                        vxt[:, ec, :],
                        wv_sb[:, ec, ht * FT : (ht + 1) * FT],
                        start=(ec == 0),
                        stop=(ec == DC - 1),
                    )
            for ht in range(2):
                vstg = stg.tile([P, FT], f32r, tag="stg", name="stg")
                nc.vector.tensor_add(
                    vstg[:], pts[ht][:], bv_sb[:, ht * FT : (ht + 1) * FT]
                )
                nc.sync.dma_start(
                    V_d.ap()[sc * P : (sc + 1) * P, ht * FT : (ht + 1) * FT],
                    vstg[:],
                )
                nc.scalar.copy(v_bf[:, sc, ht * FT : (ht + 1) * FT], vstg[:])

        # ---- attention, one 128-query chunk at a time, DESCENDING ----
        wchp.release()
        stg.release()
        psmm.release()
        with (
            tc.tile_pool(name="pbfp", bufs=2) as pbfp,
            tc.tile_pool(name="ptp", bufs=2) as ptp,
            tc.tile_pool(name="fp", bufs=2) as fp,
            tc.tile_pool(name="smp", bufs=3) as smp,
            tc.tile_pool(name="psS", bufs=4, space="PSUM") as psS,
            tc.tile_pool(name="psF", bufs=2, space="PSUM") as psF,
            tc.tile_pool(name="psT", bufs=2, space="PSUM") as psT,
        ):

            def qk_block(qi):
                n_kt = qi // 4 + 1
                if qi - 2 >= 0:
                    load_qt(qi - 2)
                qt = qt_tiles.pop(qi)
                sts = [
                    psS.tile([P, FT], f32, tag="S", name="S") for _ in range(n_kt)
                ]
                for kt in range(n_kt):
                    for hc in range(HC):
                        nc.tensor.matmul(
                            sts[kt][:],
                            qt[:, hc, :],
                            kt_sb[:, hc, kt * FT : (kt + 1) * FT],
                            start=(hc == 0),
                            stop=(hc == HC - 1),
                        )
                return sts

            def softmax_block(qi, sts):
                n_kt = len(sts)
                v = qi % 4
                nc.vector.tensor_add(sts[-1][:], sts[-1][:], mask_sb[:, v, :])
                sm = smp.tile([P, 16], f32, tag="sm", name="sm")
                for kt in range(n_kt):
                    nc.vector.reduce_max(sm[:, kt : kt + 1], sts[kt][:], axis=AX)
                negm = sm[:, 8:9]
                nc.vector.reduce_max(negm, sm[:, :n_kt], axis=AX, negate=True)
                p_bf = pbfp.tile([P, S], bf16, tag="pbf", name="pbf")
                for kt in range(n_kt):
                    nc.scalar.activation(
                        p_bf[:, kt * FT : (kt + 1) * FT],
                        sts[kt][:],
                        EXP,
                        bias=negm,
                        accum_out=sm[:, 4 + kt : 5 + kt],
                    )
                recip = sm[:, 10:11]
                if n_kt > 1:
                    nc.vector.reduce_sum(sm[:, 9:10], sm[:, 4 : 4 + n_kt], axis=AX)
                    nc.vector.reciprocal(recip, sm[:, 9:10])
                else:
                    nc.vector.reciprocal(recip, sm[:, 4:5])
                ptb = ptp.tile([P, QC, P], bf16, tag="pt", name="pt")
                for kc in range(qi + 1):
                    tp = psT.tile([P, P], bf16, tag="tp", name="tp")
                    nc.tensor.transpose(
                        tp[:], p_bf[:, kc * P : (kc + 1) * P], ident[:]
                    )
                    nc.vector.tensor_copy(ptb[:, kc, :], tp[:])
                return ptb, recip

            def pv_block(qi, ptb, recip):
                fts = [psF.tile([P, FT], f32, tag="F", name="F") for _ in range(2)]
                for kc in range(qi + 1):
                    for ht in range(2):
                        nc.tensor.matmul(
                            fts[ht][:],
                            ptb[:, kc, :],
                            v_bf[:, kc, ht * FT : (ht + 1) * FT],
                            start=(kc == 0),
                            stop=(kc == qi),
                        )
                fsb = fp.tile([P, H], f32, tag="fsb", name="fsb")
                for ht in range(2):
                    nc.vector.tensor_mul(
                        fsb[:, ht * FT : (ht + 1) * FT],
                        fts[ht][:],
                        recip.to_broadcast((P, FT)),
                    )
                nc.sync.dma_start(F_d.ap()[qi * P : (qi + 1) * P, :], fsb[:])

            pending = None
            for qi in range(QC - 1, 5, -1):
                sts = qk_block(qi)
                ptb, recip = softmax_block(qi, sts)
                if pending is not None:
                    pv_block(*pending)
                pending = (qi, ptb, recip)
            for a in (5, 3, 1):
                b = a - 1
                sts_a = qk_block(a)
                sts_b = qk_block(b)
                ptb_a, recip_a = softmax_block(a, sts_a)
                if pending is not None:
                    pv_block(*pending)
                ptb_b, recip_b = softmax_block(b, sts_b)
                pv_block(a, ptb_a, recip_a)
                pending = (b, ptb_b, recip_b)
            pv_block(*pending)
        small.release()
        vbfp.release()
        ktp.release()
        wvp.release()
        biasp.release()
        qtp.release()

    nc.compile()
    return nc


def _get_nc():
    if "nc" not in _cache:
        _cache["nc"] = _build()
    return _cache["nc"]


def _masks():
    m = np.full((4, P, FT), NEG, dtype=np.float32)
    j = np.arange(FT)[None, :]
    p = np.arange(P)[:, None]
    for v in range(4):
        m[v][j <= p + P * v] = 0.0
    return m


_last_in_maps = None


def kernel(x_batch, lin_w, lin_b, W_q, W_k, W_v):
    from concourse.bass_utils import run_bass_kernel_spmd

    nc = _get_nc()
    x_batch = np.asarray(x_batch, dtype=np.float32)
    lwT64 = np.asarray(lin_w, dtype=np.float64).T
    lb64 = np.asarray(lin_b, dtype=np.float64)
    cw = {}
    for nm, w in (("q", W_q), ("k", W_k), ("v", W_v)):
        w64 = np.asarray(w, dtype=np.float64)
        cw["W" + nm] = np.ascontiguousarray((lwT64 @ w64).astype(np.float32))
        cw["b" + nm] = (lb64 @ w64).astype(np.float32)
    bq = np.ascontiguousarray(cw["bq"].reshape(HC, P).T)  # [P, HC]
    bk = np.ascontiguousarray(cw["bk"].reshape(HC, P).T)
    bv = np.ascontiguousarray(np.tile(cw["bv"][None, :], (P, 1)))  # [P, H]
    masks = _masks()

    def sw_dPH(w):  # [D, H] -> [P, DC, H]
        return np.ascontiguousarray(w.reshape(DC, P, H).transpose(1, 0, 2))

    def sw_chunked(w):  # [D, H] -> [P, HC, DC, P]
        return np.ascontiguousarray(
            w.reshape(DC, P, HC, P).transpose(1, 2, 0, 3)
        )

    wq_sw = sw_chunked(cw["Wq"])
    wk_sw = sw_chunked(cw["Wk"])
    wv_sw = sw_dPH(cw["Wv"])
    in_maps = []
    for c in range(NB):
        xb = x_batch[c]  # [S, D]
        xT_sw = np.ascontiguousarray(
            xb.T.reshape(DC, P, S).transpose(1, 0, 2)
        )  # [P, DC, S]
        xv_sw = np.ascontiguousarray(
            xb.reshape(QC, P, DC, P).transpose(3, 0, 2, 1)
        )  # [P, QC, DC, P] : xv[p, sc, dc, j] = x[sc*128+j, dc*128+p]
        in_maps.append(
            {
                "xT": xT_sw,
                "xv": xv_sw,
                "Wq": wq_sw,
                "Wk": wk_sw,
                "Wv": wv_sw,
                "bq": bq,
                "bk": bk,
                "bv": bv,
                "masks": masks,
            }
        )
    global _last_in_maps
    _last_in_maps = in_maps
    res = run_bass_kernel_spmd(nc, in_maps, core_ids=list(range(NB)))
    F = np.stack([res.results[c]["F_out"] for c in range(NB)])
    K = np.stack([np.ascontiguousarray(res.results[c]["KT_out"].T) for c in range(NB)])
    V = np.stack([res.results[c]["V_out"] for c in range(NB)])
    cache = np.stack([K, V])
    return (F, cache)


# revision 22
# speedup vs baseline: 1.4836x; 1.0010x over previous
"""TRN2 Bass kernel for nn_Attention_1709396984084.

Reference computation (per batch element b, 8 of them -> one NeuronCore each):
    x  = x_b @ lin_w.T + lin_b          # [S, D]
    Q  = x @ W_q ; K = x @ W_k ; V = x @ W_v
    I  = Q @ K.T  (causal masked, NO 1/sqrt(d) scaling)
    F  = softmax(I) @ V
    returns (F, stack([K, V]))

Key algebraic fold (host side): xp is not an output, so
    Q = x_b @ (lin_w.T @ W_q) + (lin_b @ W_q) = x_b @ Wq' + bq
and likewise for K, V — the linear stage disappears from the device
entirely (Wq'/Wk'/Wv' and the bias vectors are precomputed in float64
on the host).

Device layout (hardcoded for S=2048, D=H=1024, B=8, batch-parallel on 8 cores):
  - host passes xT = x_b.T [D, S]; projections contract over d directly:
    Q^T/K^T [h, s] use Wq' chunks as stationary, xT as moving; V [s, h]
    uses xT chunks as stationary, Wv' as moving.
  - Q^T spills to DRAM, streamed back per 128-query chunk; K^T f32 and
    V bf16 stay resident in SBUF.
  - scores = Q^T.T @ K^T land [q, k] in PSUM; row softmax = free-dim
    reduce_max + ScalarE exp (per-partition -max bias, fused row-sum via
    accum_out); P~ cast bf16, transposed 128x128 on TensorE, then
    P~^T @ V accumulates F; 1/rowsum applied on the way out.
  - matmul dtypes: float32r (fp32 storage, ~1.5e-4 matmul rel err,
    bf16-rate at N=512) for the logit-sensitive chain; bf16 for P@V.
  - attention runs q-chunks DESCENDING with a one-chunk software pipeline
    (two-chunk for the small tail) so P@V fills the PE during softmax.
Measured: relF ~2.8e-3, relK/V ~2e-4 vs the fp32 reference.
"""

import sys

sys.path.insert(0, "/opt/trn_rl_repo")

import numpy as np

P = 128
S = 2048  # sequence length
D = 1024  # input size
H = 1024  # hidden size
FT = 512  # free-dim tile (one PSUM bank of fp32)
NB = 8  # batch == number of cores
DC = D // P  # 8 contraction chunks
HC = H // P
ST = S // FT  # 4 s-tiles
QC = S // P  # 16 query chunks
NEG = -1.0e30

_cache = {}


def _build():
    import concourse.mybir as mybir
    import concourse.tile as tile
    from concourse import bacc
    from concourse.masks import make_identity

    f32 = mybir.dt.float32
    f32r = mybir.dt.float32r
    bf16 = mybir.dt.bfloat16
    EXP = mybir.ActivationFunctionType.Exp
    AX = mybir.AxisListType.X

    nc = bacc.Bacc(None, target_bir_lowering=False)

    # all inputs pre-swizzled on the host so every DMA is long-contiguous
    # per SBUF partition (512B-run chunked loads were DMA-descriptor-bound)
    xT_d = nc.declare_dram_parameter("xT", [P, DC, S], f32r, isOutput=False)
    xv_d = nc.declare_dram_parameter("xv", [P, QC, DC, P], f32r, isOutput=False)
    wq_d = nc.declare_dram_parameter("Wq", [P, HC, DC, P], f32r, isOutput=False)
    wk_d = nc.declare_dram_parameter("Wk", [P, HC, DC, P], f32r, isOutput=False)
    wv_d = nc.declare_dram_parameter("Wv", [P, DC, H], f32r, isOutput=False)
    bq_d = nc.declare_dram_parameter("bq", [P, HC], f32, isOutput=False)
    bk_d = nc.declare_dram_parameter("bk", [P, HC], f32, isOutput=False)
    bv_d = nc.declare_dram_parameter("bv", [P, H], f32, isOutput=False)
    mask_d = nc.declare_dram_parameter("masks", [4, P, FT], f32, isOutput=False)
    F_d = nc.declare_dram_parameter("F_out", [S, H], f32, isOutput=True)
    KT_d = nc.declare_dram_parameter("KT_out", [H, S], f32r, isOutput=True)
    V_d = nc.declare_dram_parameter("V_out", [S, H], f32r, isOutput=True)
    qt_spill = nc.dram_tensor("QT_spill", [H, S], f32r)

    with tile.TileContext(nc) as tc:
        qtp = tc.alloc_tile_pool(name="qtp", bufs=2)
        biasp = tc.alloc_tile_pool(name="biasp", bufs=1)
        stg = tc.alloc_tile_pool(name="stg", bufs=3)
        wchp = tc.alloc_tile_pool(name="wchp", bufs=4)
        xtp = tc.alloc_tile_pool(name="xtp", bufs=1)
        psmm = tc.alloc_tile_pool(name="psmm", bufs=8, space="PSUM")

        xt_sb = xtp.tile([P, DC, S], f32r, tag="xt", name="xt")
        bqk_sb = biasp.tile([P, 2, HC], f32, tag="bqk", name="bqk")
        bq_sb = bqk_sb[:, 0]
        bk_sb = bqk_sb[:, 1]
        bv_sb = biasp.tile([P, H], f32, tag="bv", name="bv")

        wvp = tc.alloc_tile_pool(name="wvp", bufs=1, side="right")
        wv_sb = wvp.tile([P, DC, H], f32r, tag="wv", name="wv")
        wqp = tc.alloc_tile_pool(name="wqp", bufs=1)
        wq_sb = wqp.tile([P, HC, DC, P], f32r, tag="wq", name="wq")
        # issue order = need order: xt st0, then Wq' (hc-major), then the
        # rest of xt; Wv' is deferred until the K stage
        for dc in range(DC):
            nc.sync.dma_start(xt_sb[:, dc, 0:FT], xT_d.ap()[:, dc, 0:FT])
        for hc in range(HC):
            nc.sync.dma_start(wq_sb[:, hc], wq_d.ap()[:, hc])
            if hc == 3:
                nc.sync.dma_start(bq_sb[:], bq_d.ap())
        nc.sync.dma_start(bk_sb[:], bk_d.ap())
        nc.sync.dma_start(bv_sb[:], bv_d.ap())
        for st in range(1, ST):
            for dc in range(DC):
                nc.sync.dma_start(
                    xt_sb[:, dc, st * FT : (st + 1) * FT],
                    xT_d.ap()[:, dc, st * FT : (st + 1) * FT],
                )

        # ---- Q^T -> DRAM spill; st-outer + resident Wq' for early start ----
        for sb2 in range(2 * ST):
            st, hcb = sb2 // 2, sb2 % 2
            for hc in range(hcb * HC // 2, (hcb + 1) * HC // 2):
                pt = psmm.tile([P, FT], f32, tag="mm", name="mm")
                for dc in range(DC):
                    nc.tensor.matmul(
                        pt[:],
                        wq_sb[:, hc, dc, :],
                        xt_sb[:, dc, st * FT : (st + 1) * FT],
                        start=(dc == 0),
                        stop=(dc == DC - 1),
                    )
                qstg = stg.tile([P, FT], f32r, tag="stg", name="stg")
                nc.vector.tensor_add(
                    qstg[:], pt[:], bq_sb[:, hc : hc + 1].to_broadcast((P, FT))
                )
                nc.sync.dma_start(
                    qt_spill.ap()[hc * P : (hc + 1) * P, st * FT : (st + 1) * FT],
                    qstg[:],
                )
        wqp.release()

        # prefetch the first two attention q-chunks' Q^T columns now
        qt_tiles = {}

        def load_qt(qi):
            qt = qtp.tile([P, HC, P], f32r, tag="qt", name="qt")
            for hc in range(HC):
                nc.sync.dma_start(
                    qt[:, hc, :],
                    qt_spill.ap()[hc * P : (hc + 1) * P, qi * P : (qi + 1) * P],
                )
            qt_tiles[qi] = qt

        load_qt(QC - 1)
        load_qt(QC - 2)

        # ---- K^T resident + K cache out (streamed Wk' chunks) ----
        ktp = tc.alloc_tile_pool(name="ktp", bufs=1, side="right")
        kt_sb = ktp.tile([P, HC, S], f32r, tag="kt", name="kt")
        wk_tiles = {}

        def load_wch(hc):
            wch = wchp.tile([P, DC, P], f32r, tag="wch", name="wch")
            nc.sync.dma_start(wch[:], wk_d.ap()[:, hc])
            wk_tiles[hc] = wch

        vxt_tiles = {}

        def load_vxt(sc):
            vxt = wchp.tile([P, DC, P], f32r, tag="wch", name="wch")
            nc.sync.dma_start(vxt[:], xv_d.ap()[:, sc])
            vxt_tiles[sc] = vxt

        load_wch(0)
        load_wch(1)
        load_wch(2)
        for ec in range(DC):
            nc.sync.dma_start(wv_sb[:, ec, :], wv_d.ap()[:, ec, :])
        for hc in range(HC):
            if hc + 3 < HC:
                load_wch(hc + 3)
            elif hc >= 5:
                load_vxt(hc - 5)
            wch = wk_tiles.pop(hc)
            pts = [psmm.tile([P, FT], f32, tag="mm", name="mm") for _ in range(ST)]
            for ec in range(DC):
                for st in range(ST):
                    nc.tensor.matmul(
                        pts[st][:],
                        wch[:, ec, :],
                        xt_sb[:, ec, st * FT : (st + 1) * FT],
                        start=(ec == 0),
                        stop=(ec == DC - 1),
                    )
            for st in range(ST):
                nc.vector.tensor_add(
                    kt_sb[:, hc, st * FT : (st + 1) * FT],
                    pts[st][:],
                    bk_sb[:, hc : hc + 1].to_broadcast((P, FT)),
                )
                nc.sync.dma_start(
                    KT_d.ap()[hc * P : (hc + 1) * P, st * FT : (st + 1) * FT],
                    kt_sb[:, hc, st * FT : (st + 1) * FT],
                )

        # ---- V natural + cache out + bf16 copy ----
        # xT is streamed back from DRAM (swizzled copy) via the same chunk
        # pool the K weights used, so prefetch flows across the boundary.
        xtp.release()
        vbfp = tc.alloc_tile_pool(name="vbfp", bufs=1, side="right")
        v_bf = vbfp.tile([P, QC, H], bf16, tag="vbf", name="vbf")
        # attention constants early: masks + identity load during V
        small = tc.alloc_tile_pool(name="small", bufs=1, side="right")
        mask_sb = small.tile([P, 4, FT], f32, tag="mask", name="mask")
        for v in range(4):
            nc.sync.dma_start(mask_sb[:, v, :], mask_d.ap()[v])
        ident = small.tile([P, P], bf16, tag="ident", name="ident")
        make_identity(nc, ident[:])

        for sc in range(QC):
            if sc + 3 < QC:
                load_vxt(sc + 3)
            vxt = vxt_tiles.pop(sc)
            pts = [psmm.tile([P, FT], f32, tag="mm", name="mm") for _ in range(2)]
            for ec in range(DC):
                for ht in range(2):
                    nc.tensor.matmul(
                        pts[ht][:],
                        vxt[:, ec, :],
                        wv_sb[:, ec, ht * FT : (ht + 1) * FT],
                        start=(ec == 0),
                        stop=(ec == DC - 1),
                    )
            for ht in range(2):
                vstg = stg.tile([P, FT], f32r, tag="stg", name="stg")
                nc.vector.tensor_add(
                    vstg[:], pts[ht][:], bv_sb[:, ht * FT : (ht + 1) * FT]
                )
                nc.sync.dma_start(
                    V_d.ap()[sc * P : (sc + 1) * P, ht * FT : (ht + 1) * FT],
                    vstg[:],
                )
                nc.scalar.copy(v_bf[:, sc, ht * FT : (ht + 1) * FT], vstg[:])

        # ---- attention, one 128-query chunk at a time, DESCENDING ----
        wchp.release()
        stg.release()
        psmm.release()
        with (
            tc.tile_pool(name="pbfp", bufs=2) as pbfp,
            tc.tile_pool(name="ptp", bufs=2) as ptp,
            tc.tile_pool(name="fp", bufs=2) as fp,
            tc.tile_pool(name="smp", bufs=3) as smp,
            tc.tile_pool(name="psS", bufs=4, space="PSUM") as psS,
            tc.tile_pool(name="psF", bufs=2, space="PSUM") as psF,
            tc.tile_pool(name="psT", bufs=2, space="PSUM") as psT,
        ):

            def qk_block(qi):
                n_kt = qi // 4 + 1
                if qi - 2 >= 0:
                    load_qt(qi - 2)
                qt = qt_tiles.pop(qi)
                sts = [
                    psS.tile([P, FT], f32, tag="S", name="S") for _ in range(n_kt)
                ]
                for kt in range(n_kt):
                    for hc in range(HC):
                        nc.tensor.matmul(
                            sts[kt][:],
                            qt[:, hc, :],
                            kt_sb[:, hc, kt * FT : (kt + 1) * FT],
                            start=(hc == 0),
                            stop=(hc == HC - 1),
                        )
                return sts

            def softmax_block(qi, sts):
                n_kt = len(sts)
                v = qi % 4
                nc.vector.tensor_add(sts[-1][:], sts[-1][:], mask_sb[:, v, :])
                sm = smp.tile([P, 16], f32, tag="sm", name="sm")
                for kt in range(n_kt):
                    nc.vector.reduce_max(sm[:, kt : kt + 1], sts[kt][:], axis=AX)
                negm = sm[:, 8:9]
                nc.vector.reduce_max(negm, sm[:, :n_kt], axis=AX, negate=True)
                p_bf = pbfp.tile([P, S], bf16, tag="pbf", name="pbf")
                for kt in range(n_kt):
                    nc.scalar.activation(
                        p_bf[:, kt * FT : (kt + 1) * FT],
                        sts[kt][:],
                        EXP,
                        bias=negm,
                        accum_out=sm[:, 4 + kt : 5 + kt],
                    )
                recip = sm[:, 10:11]
                if n_kt > 1:
                    nc.vector.reduce_sum(sm[:, 9:10], sm[:, 4 : 4 + n_kt], axis=AX)
                    nc.vector.reciprocal(recip, sm[:, 9:10])
                else:
                    nc.vector.reciprocal(recip, sm[:, 4:5])
                ptb = ptp.tile([P, QC, P], bf16, tag="pt", name="pt")
                for kc in range(qi + 1):
                    tp = psT.tile([P, P], bf16, tag="tp", name="tp")
                    nc.tensor.transpose(
                        tp[:], p_bf[:, kc * P : (kc + 1) * P], ident[:]
                    )
                    nc.vector.tensor_copy(ptb[:, kc, :], tp[:])
                return ptb, recip

            def pv_block(qi, ptb, recip):
                fts = [psF.tile([P, FT], f32, tag="F", name="F") for _ in range(2)]
                for kc in range(qi + 1):
                    for ht in range(2):
                        nc.tensor.matmul(
                            fts[ht][:],
                            ptb[:, kc, :],
                            v_bf[:, kc, ht * FT : (ht + 1) * FT],
                            start=(kc == 0),
                            stop=(kc == qi),
                        )
                fsb = fp.tile([P, H], f32, tag="fsb", name="fsb")
                for ht in range(2):
                    nc.vector.tensor_mul(
                        fsb[:, ht * FT : (ht + 1) * FT],
                        fts[ht][:],
                        recip.to_broadcast((P, FT)),
                    )
                nc.sync.dma_start(F_d.ap()[qi * P : (qi + 1) * P, :], fsb[:])

            pending = None
            for qi in range(QC - 1, 5, -1):
                sts = qk_block(qi)
                ptb, recip = softmax_block(qi, sts)
                if pending is not None:
                    pv_block(*pending)
                pending = (qi, ptb, recip)
            for a in (5, 3, 1):
                b = a - 1
                sts_a = qk_block(a)
                sts_b = qk_block(b)
                ptb_a, recip_a = softmax_block(a, sts_a)
                if pending is not None:
                    pv_block(*pending)
                ptb_b, recip_b = softmax_block(b, sts_b)
                pv_block(a, ptb_a, recip_a)
                pending = (b, ptb_b, recip_b)
            pv_block(*pending)
        small.release()
        vbfp.release()
        ktp.release()
        wvp.release()
        biasp.release()
        qtp.release()

    nc.compile()
    return nc


def _get_nc():
    if "nc" not in _cache:
        _cache["nc"] = _build()
    return _cache["nc"]


def _masks():
    m = np.full((4, P, FT), NEG, dtype=np.float32)
    j = np.arange(FT)[None, :]
    p = np.arange(P)[:, None]
    for v in range(4):
        m[v][j <= p + P * v] = 0.0
    return m


_last_in_maps = None


def kernel(x_batch, lin_w, lin_b, W_q, W_k, W_v):
    from concourse.bass_utils import run_bass_kernel_spmd

    nc = _get_nc()
    x_batch = np.asarray(x_batch, dtype=np.float32)
    lwT64 = np.asarray(lin_w, dtype=np.float64).T
    lb64 = np.asarray(lin_b, dtype=np.float64)
    cw = {}
    for nm, w in (("q", W_q), ("k", W_k), ("v", W_v)):
        w64 = np.asarray(w, dtype=np.float64)
        cw["W" + nm] = np.ascontiguousarray((lwT64 @ w64).astype(np.float32))
        cw["b" + nm] = (lb64 @ w64).astype(np.float32)
    bq = np.ascontiguousarray(cw["bq"].reshape(HC, P).T)  # [P, HC]
    bk = np.ascontiguousarray(cw["bk"].reshape(HC, P).T)
    bv = np.ascontiguousarray(np.tile(cw["bv"][None, :], (P, 1)))  # [P, H]
    masks = _masks()

    def sw_dPH(w):  # [D, H] -> [P, DC, H]
        return np.ascontiguousarray(w.reshape(DC, P, H).transpose(1, 0, 2))

    def sw_chunked(w):  # [D, H] -> [P, HC, DC, P]
        return np.ascontiguousarray(
            w.reshape(DC, P, HC, P).transpose(1, 2, 0, 3)
        )

    wq_sw = sw_chunked(cw["Wq"])
    wk_sw = sw_chunked(cw["Wk"])
    wv_sw = sw_dPH(cw["Wv"])
    in_maps = []
    for c in range(NB):
        xb = x_batch[c]  # [S, D]
        xT_sw = np.ascontiguousarray(
            xb.T.reshape(DC, P, S).transpose(1, 0, 2)
        )  # [P, DC, S]
        xv_sw = np.ascontiguousarray(
            xb.reshape(QC, P, DC, P).transpose(3, 0, 2, 1)
        )  # [P, QC, DC, P] : xv[p, sc, dc, j] = x[sc*128+j, dc*128+p]
        in_maps.append(
            {
                "xT": xT_sw,
                "xv": xv_sw,
                "Wq": wq_sw,
                "Wk": wk_sw,
                "Wv": wv_sw,
                "bq": bq,
                "bk": bk,
                "bv": bv,
                "masks": masks,
            }
        )
    global _last_in_maps
    _last_in_maps = in_maps
    res = run_bass_kernel_spmd(nc, in_maps, core_ids=list(range(NB)))
    F = np.stack([res.results[c]["F_out"] for c in range(NB)])
    K = np.stack([np.ascontiguousarray(res.results[c]["KT_out"].T) for c in range(NB)])
    V = np.stack([res.results[c]["V_out"] for c in range(NB)])
    cache = np.stack([K, V])
    return (F, cache)
